# revision 6
# baseline (speedup 1.0000x reference)
"""GCN message-passing kernel for 8 Trainium2 NeuronCores (Bass/Tile).

Computes (matching the jax reference):
    h = x @ W_conv                      [N, H]
    node_embed = leaky_relu(D^-1/2 (A+I) D^-1/2 h + b_conv)
    out = sigmoid(leaky(cat(e[i], e[j]) @ W1 + b1) @ W2 + b2)

Distribution: nodes dst-sharded over 8 cores. Scaled features g = dinv*h are
AllGathered in two halves (each core's shard split lo/hi) so the edge
aggregation phase C1 (lo sources) starts while the hi half is still in
flight. Per-edge source rows are fetched with dma_gather (pair-packed fp16
rows) spread across the 4 SWDGE queues; scatter-add onto destination tiles
runs on the TensorEngine via one-hot matmuls. The e AllGather is likewise
split and overlapped with the tail of phase C2; the pair-MLP head reuses the
gather + one-hot machinery on e_full.
"""

import re

import numpy as np

import concourse.bass as bass
import concourse.bacc as bacc
import concourse.mybir as mybir
import concourse.tile as tile
from concourse import library_config
from concourse.bass_utils import run_bass_kernel_spmd

NC = 8
N_NODES = 100000
F_IN = 256
H = 64
NEG = 0.01

P = 128                    # partitions / tile height
TILES = 98                 # node tiles per core
SHARD = TILES * P          # 12544 nodes per core
NPAD = NC * SHARD          # 100352
HALF_TILES = 49
HALF = HALF_TILES * P      # 6272 nodes = lo region of each shard
REG = NC * HALF            # 50176 nodes per region
RPAIR = REG // 2           # 25088 pair rows per region (int16-addressable)
GROUP = 4                  # dst node tiles per edge gather group
NGROUP = (TILES + GROUP - 1) // GROUP
PTILES = 32                # pair slot tiles per core (2*2048/128)
AG2_TRIGGER_GROUP = 16     # emit e-lo AllGather after this C2 group


def _node_addr(n):
    """node id -> (region, pair_row_in_region, parity)."""
    c = n // SHARD
    l = n % SHARD
    r = (l >= HALF).astype(np.int64)
    pos = c * HALF + (l - r * HALF)
    return r, pos >> 1, pos & 1


def _wrap_idx(idx):
    """int array [W] (W % 16 == 0) -> [128, W//16] int16 wrapped/replicated."""
    w = idx.reshape(-1, 16).T.astype(np.int16)
    return np.tile(w, (8, 1))


def _prep(inputs):
    x = np.asarray(inputs["x"], np.float32)
    edge_index = np.asarray(inputs["edge_index"], np.int64)
    index = np.asarray(inputs["index"], np.int64)
    W_conv = np.asarray(inputs["W_conv"], np.float32)
    b_conv = np.asarray(inputs["b_conv"], np.float32)
    W1 = np.asarray(inputs["W1"], np.float32)
    b1 = np.asarray(inputs["b1"], np.float32)
    W2 = np.asarray(inputs["W2"], np.float32)
    b2 = np.asarray(inputs["b2"], np.float32)

    n = x.shape[0]
    src = edge_index[0].astype(np.int64)
    dst = edge_index[1].astype(np.int64)
    loops = np.arange(n, dtype=np.int64)
    src = np.concatenate([src, loops])
    dst = np.concatenate([dst, loops])

    deg = np.bincount(dst, minlength=NPAD).astype(np.float32)
    deg[n:] = 1.0
    dinv = np.zeros(NPAD, np.float32)
    dinv[deg > 0] = 1.0 / np.sqrt(deg[deg > 0])

    # pair stream: per core PB pairs; slots [xi: 0..PB) [xj: PB..2PB)
    B = index.shape[0]
    PB = B // NC
    PCH = PB // P
    pair_global = np.arange(B, dtype=np.int64)
    pcore = pair_global // PB
    plocal = pair_global % PB
    s_core = np.concatenate([pcore, pcore])
    s_slot = np.concatenate([plocal, PB + plocal])
    s_node = np.concatenate([index[:, 0], index[:, 1]]).astype(np.int64)

    e_percore, e_shape = _sched_all(
        core=dst // SHARD, tl=(dst % SHARD) // P, loc=dst % P, node=src,
        ntiles=TILES, group_sz=GROUP)
    p_percore, p_shape = _sched_all(
        core=s_core, tl=s_slot // P, loc=s_slot % P, node=s_node,
        ntiles=2 * PCH, group_sz=GROUP)

    xpad = np.zeros((NPAD, F_IN), np.float32)
    xpad[:n] = x
    xT = xpad.T.astype(np.float16)
    xT_shards = [
        np.ascontiguousarray(
            xT[:, c * SHARD:(c + 1) * SHARD].reshape(2, P, SHARD).transpose(1, 0, 2)
        ) for c in range(NC)
    ]
    dinv_sb = [
        np.ascontiguousarray(
            dinv[c * SHARD:(c + 1) * SHARD].reshape(TILES, P).T
        ) for c in range(NC)
    ]

    consts = {
        "wc": np.ascontiguousarray(
            W_conv.reshape(2, P, H).transpose(1, 0, 2)).astype(np.float16),
        "bconvb": np.broadcast_to(b_conv, (P, H)).astype(np.float32).copy(),
        "iota": np.broadcast_to(np.arange(P, dtype=np.float16), (P, P)).copy(),
        "ident": np.eye(P, dtype=np.float16),
        "w1": W1.astype(np.float16),
        "b1": b1.reshape(16, 1).astype(np.float32),
        "w2": W2.astype(np.float32),
        "b2t": b2.reshape(1, 1).astype(np.float32),
    }
    sched = {"edge": e_shape, "pair": p_shape, "PCH": PCH}
    in_maps = []
    for c in range(NC):
        m = {
            "xt": xT_shards[c],
            "dinv": dinv_sb[c],
            "eloc0": e_percore[c][0]["loc"], "eidx0": e_percore[c][0]["idx"],
            "eloc1": e_percore[c][1]["loc"], "eidx1": e_percore[c][1]["idx"],
            "ploc0": p_percore[c][0]["loc"], "pidx0": p_percore[c][0]["idx"],
            "ploc1": p_percore[c][1]["loc"], "pidx1": p_percore[c][1]["idx"],
        }
        m.update(consts)
        in_maps.append(m)
    return in_maps, sched


def _sched_all(core, tl, loc, node, ntiles, group_sz):
    """Schedule with K maxes over all cores, per-core idx/loc tables."""
    region, prow, parity = _node_addr(node)
    ngroups = (ntiles + group_sz - 1) // group_sz
    grp = tl // group_sz

    shape_phases = []
    percore = [[None, None] for _ in range(NC)]
    for r in (0, 1):
        sel = region == r
        s_core, s_tl, s_loc = core[sel], tl[sel], loc[sel]
        s_grp, s_q, s_prow = grp[sel], parity[sel], prow[sel]

        tid = ((s_core * ngroups + s_grp) * 2 + s_q) * ntiles + s_tl
        n_bins = NC * ngroups * 2 * ntiles
        cnt = np.bincount(tid, minlength=n_bins).reshape(NC, ngroups, 2, ntiles)
        K = (cnt + P - 1) // P
        K = K.max(axis=0)
        for t in range(ntiles):
            g = t // group_sz
            if K[g, :, t].sum() == 0:
                K[g, 0, t] = 1

        chunk_meta = []
        win_meta = []
        slot_base = np.zeros((ngroups, 2, ntiles), np.int64)
        for g in range(ngroups):
            t0, t1 = g * group_sz, min((g + 1) * group_sz, ntiles)
            for q in range(2):
                c0 = len(chunk_meta)
                for t in range(t0, t1):
                    slot_base[g, q, t] = len(chunk_meta) * P
                    for _ in range(K[g, q, t]):
                        chunk_meta.append((g, q, t))
                win_meta.append((g, q, c0, len(chunk_meta) - c0))
        totchunks = len(chunk_meta)
        totidx = totchunks * P

        shape_phases.append({
            "chunk_meta": chunk_meta,
            "win_meta": win_meta,
            "totchunks": totchunks,
            "totidx": totidx,
            "ntiles": ntiles,
            "ngroups": ngroups,
            "group_sz": group_sz,
        })

        for c in range(NC):
            msk = s_core == c
            c_tl, c_loc = s_tl[msk], s_loc[msk]
            c_grp, c_q, c_prow = s_grp[msk], s_q[msk], s_prow[msk]
            loc_arr = np.full(totidx, 255, np.int64)
            pidx_arr = np.zeros(totidx, np.int64)
            order = np.lexsort((c_tl, c_q, c_grp))
            so_tid = (c_grp[order] * 2 + c_q[order]) * ntiles + c_tl[order]
            if len(so_tid):
                starts = np.r_[0, np.flatnonzero(np.diff(so_tid)) + 1]
                run_ids = np.zeros(len(so_tid), np.int64)
                run_ids[starts[1:]] = 1
                run_ids = np.cumsum(run_ids)
                rank = np.arange(len(so_tid)) - starts[run_ids]
                slot = slot_base[c_grp[order], c_q[order], c_tl[order]] + rank
                loc_arr[slot] = c_loc[order]
                pidx_arr[slot] = c_prow[order]
            percore[c][r] = {
                "loc": np.ascontiguousarray(
                    loc_arr.reshape(totchunks, P).T.astype(np.float16)),
                "idx": np.ascontiguousarray(_wrap_idx(pidx_arr)),
            }
    return percore, shape_phases


def _build(sched, debug=False):
    dt = mybir.dt
    e_ph = sched["edge"]
    p_ph = sched["pair"]
    PCH = sched["PCH"]

    nc = bacc.Bacc("TRN2", target_bir_lowering=False, debug=False,
                   enable_asserts=False, num_devices=NC, num_swdge_queues=4)

    xt_in = nc.dram_tensor("xt", [P, 2, SHARD], dt.float16, kind="ExternalInput")
    dinv_in = nc.dram_tensor("dinv", [P, TILES], dt.float32, kind="ExternalInput")
    eloc_in = [nc.dram_tensor(f"eloc{r}", [P, e_ph[r]["totchunks"]], dt.float16,
                              kind="ExternalInput") for r in (0, 1)]
    eidx_in = [nc.dram_tensor(f"eidx{r}", [P, e_ph[r]["totidx"] // 16], dt.int16,
                              kind="ExternalInput") for r in (0, 1)]
    ploc_in = [nc.dram_tensor(f"ploc{r}", [P, p_ph[r]["totchunks"]], dt.float16,
                              kind="ExternalInput") for r in (0, 1)]
    pidx_in = [nc.dram_tensor(f"pidx{r}", [P, p_ph[r]["totidx"] // 16], dt.int16,
                              kind="ExternalInput") for r in (0, 1)]
    wc_in = nc.dram_tensor("wc", [P, 2, H], dt.float16, kind="ExternalInput")
    bconvb_in = nc.dram_tensor("bconvb", [P, H], dt.float32, kind="ExternalInput")
    iota_in = nc.dram_tensor("iota", [P, P], dt.float16, kind="ExternalInput")
    ident_in = nc.dram_tensor("ident", [P, P], dt.float16, kind="ExternalInput")
    w1_in = nc.dram_tensor("w1", [P, 16], dt.float16, kind="ExternalInput")
    b1_in = nc.dram_tensor("b1", [16, 1], dt.float32, kind="ExternalInput")
    w2_in = nc.dram_tensor("w2", [16, 1], dt.float32, kind="ExternalInput")
    b2_in = nc.dram_tensor("b2t", [1, 1], dt.float32, kind="ExternalInput")
    outp = nc.dram_tensor("out", [PCH * P, 1], dt.float32, kind="ExternalOutput")

    g_shard = [nc.dram_tensor(f"g_shard{r}", [HALF, H], dt.float16)
               for r in (0, 1)]
    g_full = [nc.dram_tensor(f"g_full{r}", [REG, H], dt.float16,
                             addr_space="Shared") for r in (0, 1)]
    e_shard = [nc.dram_tensor(f"e_shard{r}", [HALF, H], dt.float16)
               for r in (0, 1)]
    e_full = [nc.dram_tensor(f"e_full{r}", [REG, H], dt.float16,
                             addr_space="Shared") for r in (0, 1)]

    g_pairs = [g_full[r][:, :].rearrange("(r two) f -> r (two f)", two=2)
               for r in (0, 1)]
    e_pairs = [e_full[r][:, :].rearrange("(r two) f -> r (two f)", two=2)
               for r in (0, 1)]

    with tile.TileContext(nc) as tc:
        nc.gpsimd.load_library(library_config.mlp)

        with (
            tc.tile_pool(name="const", bufs=1) as cpool,
            tc.tile_pool(name="idxp", bufs=1) as idxp,
        ):
            wc_sb = cpool.tile([P, 2, H], dt.float16)
            nc.sync.dma_start(wc_sb[:], wc_in[:, :, :])
            bconvb = cpool.tile([P, H], dt.float32)
            nc.sync.dma_start(bconvb[:], bconvb_in[:, :])
            iota = cpool.tile([P, P], dt.float16)
            nc.sync.dma_start(iota[:], iota_in[:, :])
            ident = cpool.tile([P, P], dt.float16)
            nc.sync.dma_start(ident[:], ident_in[:, :])
            w1_sb = cpool.tile([P, 16], dt.float16)
            nc.sync.dma_start(w1_sb[:], w1_in[:, :])
            b1_sb = cpool.tile([16, 1], dt.float32)
            nc.sync.dma_start(b1_sb[:], b1_in[:, :])
            w2_sb = cpool.tile([16, 1], dt.float32)
            nc.sync.dma_start(w2_sb[:], w2_in[:, :])
            b2_sb = cpool.tile([1, 1], dt.float32)
            nc.sync.dma_start(b2_sb[:], b2_in[:, :])
            dinv_sb = cpool.tile([P, TILES], dt.float32)
            nc.sync.dma_start(dinv_sb[:], dinv_in[:, :])
            eloc_sb = [cpool.tile([P, e_ph[r]["totchunks"]], dt.float16,
                                  name=f"eloc{r}") for r in (0, 1)]
            eidx_sb = [idxp.tile([P, e_ph[r]["totidx"] // 16], dt.int16,
                                 name=f"eidx{r}") for r in (0, 1)]
            # prefetch phase-0 tables up front; phase-1 tables stream later
            nc.sync.dma_start(eloc_sb[0][:], eloc_in[0][:, :])
            nc.sync.dma_start(eidx_sb[0][:], eidx_in[0][:, :])

            # ---------------- phase A: g = (x @ W) * dinv ----------------
            XBLK = 7
            with (
                tc.tile_pool(name="xtp", bufs=2) as xtp,
                tc.tile_pool(name="hps", bufs=4, space="PSUM") as hps,
                tc.tile_pool(name="gsb", bufs=1) as gsbp,
            ):
                g_sb = gsbp.tile([P, TILES, H], dt.float16)
                for blk in range((TILES + XBLK - 1) // XBLK):
                    t0, t1 = blk * XBLK, min((blk + 1) * XBLK, TILES)
                    xt_sb = xtp.tile([P, 2, (t1 - t0) * P], dt.float16, tag="xt")
                    nc.sync.dma_start(xt_sb[:], xt_in[:, :, t0 * P: t1 * P])
                    for t in range(t0, t1):
                        h_ps = hps.tile([P, H], dt.float32)
                        for k in range(2):
                            nc.tensor.matmul(
                                h_ps[:],
                                lhsT=xt_sb[:, k, (t - t0) * P:(t - t0 + 1) * P],
                                rhs=wc_sb[:, k, :],
                                start=(k == 0), stop=(k == 1),
                            )
                        nc.vector.tensor_scalar(
                            g_sb[:, t, :], h_ps[:], dinv_sb[:, t:t + 1], None,
                            mybir.AluOpType.mult,
                        )
                    if t1 == HALF_TILES:
                        nc.sync.dma_start(
                            g_shard[0][:, :].rearrange("(t p) f -> p t f", p=P),
                            g_sb[:, :HALF_TILES, :])
                        nc.gpsimd.collective_compute(
                            "AllGather", mybir.AluOpType.bypass,
                            replica_groups=[list(range(NC))],
                            ins=[g_shard[0][:, :].opt()],
                            outs=[g_full[0][:, :].opt()],
                        )
                nc.sync.dma_start(
                    g_shard[1][:, :].rearrange("(t p) f -> p t f", p=P),
                    g_sb[:, HALF_TILES:, :])
                nc.gpsimd.collective_compute(
                    "AllGather", mybir.AluOpType.bypass,
                    replica_groups=[list(range(NC))],
                    ins=[g_shard[1][:, :].opt()],
                    outs=[g_full[1][:, :].opt()],
                )

            # stream phase-1 edge tables while C1 runs
            nc.sync.dma_start(eloc_sb[1][:], eloc_in[1][:, :])
            nc.sync.dma_start(eidx_sb[1][:], eidx_in[1][:, :])

            # ---------------- phases C1/C2: edge aggregation ----------------
            with (
                tc.tile_pool(name="emsg", bufs=3) as msgp,
                tc.tile_pool(name="eoh", bufs=2) as ohp,
                tc.tile_pool(name="eacc", bufs=2, space="PSUM") as accp,
                tc.tile_pool(name="partial", bufs=1) as partp,
                tc.tile_pool(name="epost", bufs=4) as postp,
                tc.tile_pool(name="eemb", bufs=3) as embp,
            ):
                partial = partp.tile([P, TILES, H], dt.float32)

                for r in (0, 1):
                    ph = e_ph[r]
                    chunk_meta = ph["chunk_meta"]
                    first_chunk, last_chunk = {}, {}
                    for ci, (g, q, t) in enumerate(chunk_meta):
                        first_chunk.setdefault(t, ci)
                        last_chunk[t] = ci

                    emb_stage = None
                    emb_base = 0
                    for g in range(NGROUP):
                        t0 = g * GROUP
                        t1 = min(t0 + GROUP, TILES)
                        acc_tiles = {}
                        wins = [w for w in ph["win_meta"] if w[0] == g]
                        # gathers first (both parities) to keep queues busy
                        msgs = {}
                        for (gg, q, c0, nch) in wins:
                            if nch == 0:
                                continue
                            msg = msgp.tile([P, nch, P], dt.float16,
                                            tag=f"emsg{q}",
                                            name=f"emsg_r{r}g{g}q{q}")
                            nc.gpsimd.dma_gather(
                                msg[:], g_pairs[r],
                                eidx_sb[r][:, c0 * P // 16:(c0 + nch) * P // 16],
                                nch * P, nch * P, P,
                                single_packet=False, queue_num=0)
                            msgs[q] = (msg, c0, nch)
                        for (gg, q, c0, nch) in wins:
                            if nch == 0:
                                continue
                            msg, _, _ = msgs[q]
                            oh = ohp.tile([P, nch, P], dt.float16,
                                          tag=f"eoh{q}", name=f"eoh_r{r}g{g}q{q}")
                            nc.vector.tensor_tensor(
                                oh[:],
                                eloc_sb[r][:, c0:c0 + nch].unsqueeze(2)
                                    .to_broadcast([P, nch, P]),
                                iota[:, :].unsqueeze(1).to_broadcast([P, nch, P]),
                                mybir.AluOpType.is_equal,
                            )
                            for ci in range(c0, c0 + nch):
                                _, qq, t = chunk_meta[ci]
                                if t not in acc_tiles:
                                    acc_tiles[t] = accp.tile(
                                        [P, H], dt.float32, tag=f"eacc{t % GROUP}",
                                        name=f"eacc_r{r}t{t}")
                                nc.tensor.matmul(
                                    acc_tiles[t][:],
                                    lhsT=oh[:, ci - c0, :],
                                    rhs=msg[:, ci - c0, qq * H:(qq + 1) * H],
                                    start=(ci == first_chunk[t]),
                                    stop=(ci == last_chunk[t]),
                                )
                        if r == 0:
                            for t in range(t0, t1):
                                nc.vector.tensor_copy(
                                    partial[:, t, :], acc_tiles.pop(t)[:])
                        else:
                            if emb_stage is None:
                                emb_stage = embp.tile(
                                    [P, GROUP, H], dt.float16, tag="emb",
                                    name=f"emb_g{g}")
                                emb_base = t0
                            for t in range(t0, t1):
                                a = acc_tiles.pop(t)
                                e1 = postp.tile([P, H], dt.float32, tag="e1",
                                                name=f"e1_{t}")
                                nc.vector.tensor_tensor(
                                    e1[:], a[:], partial[:, t, :],
                                    mybir.AluOpType.add)
                                nc.vector.tensor_scalar(
                                    e1[:], e1[:], dinv_sb[:, t:t + 1], None,
                                    mybir.AluOpType.mult)
                                nc.vector.tensor_tensor(
                                    e1[:], e1[:], bconvb[:], mybir.AluOpType.add)
                                m = postp.tile([P, H], dt.float32, tag="m",
                                               name=f"m_{t}")
                                nc.scalar.activation(
                                    m[:], e1[:],
                                    mybir.ActivationFunctionType.Copy,
                                    bias=0.0, scale=NEG)
                                nc.vector.tensor_tensor(
                                    emb_stage[:, t - emb_base, :], e1[:], m[:],
                                    mybir.AluOpType.max)
                                # flush stage at region boundary or group end
                                flush = (t == t1 - 1) or (t == HALF_TILES - 1)
                                if flush:
                                    lo = emb_base
                                    hi = t + 1
                                    reg = 0 if t < HALF_TILES else 1
                                    base = lo - (0 if reg == 0 else HALF_TILES)
                                    nc.sync.dma_start(
                                        e_shard[reg][base * P:(base + hi - lo) * P, :]
                                        .rearrange("(t p) f -> p t f", p=P),
                                        emb_stage[:, lo - emb_base:hi - emb_base, :])
                                    if hi - 1 == HALF_TILES - 1:
                                        nc.gpsimd.collective_compute(
                                            "AllGather", mybir.AluOpType.bypass,
                                            replica_groups=[list(range(NC))],
                                            ins=[e_shard[0][:, :].opt()],
                                            outs=[e_full[0][:, :].opt()],
                                        )
                                    if t < t1 - 1:
                                        emb_stage = embp.tile(
                                            [P, GROUP, H], dt.float16, tag="emb",
                                            name=f"emb_g{g}b")
                                        emb_base = t + 1
                            emb_stage = None

                nc.gpsimd.collective_compute(
                    "AllGather", mybir.AluOpType.bypass,
                    replica_groups=[list(range(NC))],
                    ins=[e_shard[1][:, :].opt()],
                    outs=[e_full[1][:, :].opt()],
                )

            # ---------------- phase D: pair gather + MLP ----------------
            with (
                tc.tile_pool(name="pconst", bufs=1) as pcpool,
                tc.tile_pool(name="pmsg", bufs=2) as pmsgp,
                tc.tile_pool(name="poh", bufs=2) as pohp,
                tc.tile_pool(name="pacc", bufs=1, space="PSUM") as paccp,
                tc.tile_pool(name="pxs", bufs=1) as pxsp,
                tc.tile_pool(name="ptps", bufs=2, space="PSUM") as ptps,
                tc.tile_pool(name="pzps", bufs=1, space="PSUM") as pzps,
                tc.tile_pool(name="pops", bufs=1, space="PSUM") as pops,
                tc.tile_pool(name="psb", bufs=4) as psbp,
            ):
                ploc_sb = [pcpool.tile([P, p_ph[r]["totchunks"]], dt.float16,
                                       name=f"ploc{r}") for r in (0, 1)]
                pidx_sb = [pcpool.tile([P, p_ph[r]["totidx"] // 16], dt.int16,
                                       name=f"pidx{r}") for r in (0, 1)]
                for r in (0, 1):
                    nc.sync.dma_start(ploc_sb[r][:], ploc_in[r][:, :])
                    nc.sync.dma_start(pidx_sb[r][:], pidx_in[r][:, :])
                xs_sb = pxsp.tile([P, 2 * PCH, H], dt.float16)

                pgroups = (2 * PCH + GROUP - 1) // GROUP
                for r in (0, 1):
                    ph = p_ph[r]
                    chunk_meta = ph["chunk_meta"]
                    first_chunk, last_chunk = {}, {}
                    for ci, (g, q, t) in enumerate(chunk_meta):
                        first_chunk.setdefault(t, ci)
                        last_chunk[t] = ci
                    for g in range(pgroups):
                        t0, t1 = g * GROUP, min((g + 1) * GROUP, 2 * PCH)
                        acc_tiles = {}
                        wins = [w for w in ph["win_meta"] if w[0] == g]
                        msgs = {}
                        for (gg, q, c0, nch) in wins:
                            if nch == 0:
                                continue
                            msg = pmsgp.tile([P, nch, P], dt.float16,
                                             tag=f"pmsg{q}",
                                             name=f"pmsg_r{r}g{g}q{q}")
                            nc.gpsimd.dma_gather(
                                msg[:], e_pairs[r],
                                pidx_sb[r][:, c0 * P // 16:(c0 + nch) * P // 16],
                                nch * P, nch * P, P,
                                single_packet=False, queue_num=0)
                            msgs[q] = msg
                        for (gg, q, c0, nch) in wins:
                            if nch == 0:
                                continue
                            msg = msgs[q]
                            oh = pohp.tile([P, nch, P], dt.float16,
                                           tag=f"poh{q}",
                                           name=f"poh_r{r}g{g}q{q}")
                            nc.vector.tensor_tensor(
                                oh[:],
                                ploc_sb[r][:, c0:c0 + nch].unsqueeze(2)
                                    .to_broadcast([P, nch, P]),
                                iota[:, :].unsqueeze(1).to_broadcast([P, nch, P]),
                                mybir.AluOpType.is_equal,
                            )
                            for ci in range(c0, c0 + nch):
                                _, qq, t = chunk_meta[ci]
                                if t not in acc_tiles:
                                    acc_tiles[t] = paccp.tile(
                                        [P, H], dt.float32, tag=f"pacc{t % GROUP}",
                                        name=f"pacc_r{r}t{t}")
                                nc.tensor.matmul(
                                    acc_tiles[t][:],
                                    lhsT=oh[:, ci - c0, :],
                                    rhs=msg[:, ci - c0, qq * H:(qq + 1) * H],
                                    start=(ci == first_chunk[t]),
                                    stop=(ci == last_chunk[t]),
                                )
                        for t in range(t0, t1):
                            a = acc_tiles.pop(t)
                            if r == 0:
                                nc.vector.tensor_copy(xs_sb[:, t, :], a[:])
                            else:
                                nc.vector.tensor_tensor(
                                    xs_sb[:, t, :], xs_sb[:, t, :], a[:],
                                    mybir.AluOpType.add)

                for k in range(PCH):
                    xt_ps = ptps.tile([P, P], dt.float16)
                    nc.tensor.transpose(xt_ps[0:H, :], xs_sb[:, k, :], ident[:])
                    nc.tensor.transpose(xt_ps[H:P, :], xs_sb[:, PCH + k, :],
                                        ident[:])
                    xijt = psbp.tile([P, P], dt.float16, tag="xijt")
                    nc.vector.tensor_copy(xijt[:], xt_ps[:])
                    z_ps = pzps.tile([16, P], dt.float32)
                    nc.tensor.matmul(z_ps[:], lhsT=w1_sb[:], rhs=xijt[:],
                                     start=True, stop=True)
                    zb = psbp.tile([16, P], dt.float32, tag="zb")
                    nc.vector.tensor_scalar(
                        zb[:], z_ps[:], b1_sb[:, 0:1], None, mybir.AluOpType.add)
                    m2 = psbp.tile([16, P], dt.float32, tag="m2")
                    nc.scalar.activation(
                        m2[:], zb[:], mybir.ActivationFunctionType.Copy,
                        bias=0.0, scale=NEG)
                    z2 = psbp.tile([16, P], dt.float32, tag="z2")
                    nc.vector.tensor_tensor(z2[:], zb[:], m2[:],
                                            mybir.AluOpType.max)
                    o_ps = pops.tile([1, P], dt.float32)
                    nc.tensor.matmul(o_ps[:], lhsT=w2_sb[:], rhs=z2[:],
                                     start=True, stop=True)
                    osb = psbp.tile([1, P], dt.float32, tag="osb")
                    nc.scalar.activation(
                        osb[:], o_ps[:], mybir.ActivationFunctionType.Sigmoid,
                        bias=b2_sb[:, 0:1], scale=1.0)
                    nc.sync.dma_start(
                        outp[k * P:(k + 1) * P, :].rearrange("r one -> one r"),
                        osb[0:1, :])

    # align each gather's SWDGE queue with its Tile-assigned DMA lane so
    # semaphore<->queue locking stays consistent (4-way parallel desc gen)
    for blk in nc.m.functions[0].blocks:
        for inst in blk.instructions:
            if isinstance(inst, mybir.InstDMAGatherAnt):
                si = inst.sync_info
                for u in (si.on_update if si else []):
                    mm = re.match(r"DMASW(\d+)_", u.ant_name or "")
                    if mm:
                        inst.queue_num = int(mm.group(1)) % 4
                        break

    nc.compile()
    return nc


def kernel(**inputs) -> np.ndarray:
    in_maps, sched = _prep(inputs)
    nc = _build(sched)
    res = run_bass_kernel_spmd(nc, in_maps, list(range(NC)))
    out = np.concatenate([res.results[c]["out"] for c in range(NC)], axis=0)
    return out.astype(np.float32)


# revision 7
# speedup vs baseline: 1.0740x; 1.0740x over previous
"""GCN message-passing kernel for 8 Trainium2 NeuronCores (Bass/Tile).

Computes (matching the jax reference):
    h = x @ W_conv                      [N, H]
    node_embed = leaky_relu(D^-1/2 (A+I) D^-1/2 h + b_conv)
    out = sigmoid(leaky(cat(e[i], e[j]) @ W1 + b1) @ W2 + b2)

Distribution: nodes dst-sharded over 8 cores. Scaled features g = dinv*h are
AllGathered in two halves (each shard split lo/hi) so edge windows sourcing
the lo half start while the hi half is in flight. Per-edge source rows are
fetched with dma_gather (pair-packed fp16 rows) spread across the 4 SWDGE
queues; scatter-add onto destination tiles runs on the TensorEngine via
one-hot matmuls whose one-hot operands are HOST-precomputed (dinv[dst]
folded in) and streamed from HBM — the DVE is kept entirely idle during
gather phases because GpSimd SWDGE and DVE perf-mode ops take an exclusive
full-instruction lock on a shared SBUF port pair. Per-tile bias lands via a
rank-1 matmul; leaky_relu runs on the ACT engine (own ports). The e
AllGather is likewise split and overlapped with the tail of phase C.
"""

import re

import numpy as np

import concourse.bass as bass
import concourse.bacc as bacc
import concourse.mybir as mybir
import concourse.tile as tile
from concourse import library_config
from concourse.bass_utils import run_bass_kernel_spmd

NC = 8
N_NODES = 100000
F_IN = 256
H = 64
NEG = 0.01

P = 128                    # partitions / tile height
TILES = 98                 # node tiles per core
SHARD = TILES * P          # 12544 nodes per core
NPAD = NC * SHARD          # 100352
HALF_TILES = 49
HALF = HALF_TILES * P      # 6272 nodes = lo region of each shard
REG = NC * HALF            # 50176 nodes per region
RPAIR = REG // 2           # 25088 pair rows per region (int16-addressable)
GROUP = 4                  # dst node tiles per edge gather group
NGROUP = (TILES + GROUP - 1) // GROUP
AG2LO_EMIT_GROUP = 15      # emit e-lo AllGather before this C group


def _node_addr(n):
    """node id -> (region, pair_row_in_region, parity)."""
    c = n // SHARD
    l = n % SHARD
    r = (l >= HALF).astype(np.int64)
    pos = c * HALF + (l - r * HALF)
    return r, pos >> 1, pos & 1


def _wrap_idx(idx):
    """int array [W] (W % 16 == 0) -> [128, W//16] int16 wrapped/replicated."""
    w = idx.reshape(-1, 16).T.astype(np.int16)
    return np.tile(w, (8, 1))


def _sched_all(core, tl, loc, node, ntiles, group_sz, ohval):
    """Single-pass schedule: bins (g, r, q, t), windows (g, r, q), chunk
    order g -> r -> q -> t -> k. K maxes over cores (SPMD-uniform shapes).

    ohval[item] = value placed in the one-hot at the item's (slot, loc).
    Returns (percore list of {idx, oh}, shape dict).
    """
    region, prow, parity = _node_addr(node)
    ngroups = (ntiles + group_sz - 1) // group_sz
    grp = tl // group_sz

    tid = (((core * ngroups + grp) * 2 + region) * 2 + parity) * ntiles + tl
    n_bins = NC * ngroups * 2 * 2 * ntiles
    cnt = np.bincount(tid, minlength=n_bins).reshape(NC, ngroups, 2, 2, ntiles)
    K = (cnt + P - 1) // P
    K = K.max(axis=0)                          # [ngroups, 2, 2, ntiles]
    for t in range(ntiles):
        g = t // group_sz
        if K[g, :, :, t].sum() == 0:
            K[g, 0, 0, t] = 1                  # keep acc defined

    chunk_meta = []                            # (g, r, q, t)
    win_meta = []                              # (g, r, q, c0, nch)
    slot_base = np.zeros((ngroups, 2, 2, ntiles), np.int64)
    for g in range(ngroups):
        t0, t1 = g * group_sz, min((g + 1) * group_sz, ntiles)
        for r in (0, 1):
            for q in (0, 1):
                c0 = len(chunk_meta)
                for t in range(t0, t1):
                    slot_base[g, r, q, t] = len(chunk_meta) * P
                    for _ in range(K[g, r, q, t]):
                        chunk_meta.append((g, r, q, t))
                win_meta.append((g, r, q, c0, len(chunk_meta) - c0))
    totchunks = len(chunk_meta)
    totidx = totchunks * P
    chunk_tile = np.array([t for (_, _, _, t) in chunk_meta], np.int64)

    shape = {
        "chunk_meta": chunk_meta,
        "win_meta": win_meta,
        "totchunks": totchunks,
        "totidx": totidx,
        "ntiles": ntiles,
        "ngroups": ngroups,
        "group_sz": group_sz,
    }

    percore = []
    for c in range(NC):
        msk = core == c
        c_tl, c_loc = tl[msk], loc[msk]
        c_grp, c_r, c_q = grp[msk], region[msk], parity[msk]
        c_prow, c_val = prow[msk], ohval[msk]
        loc_arr = np.full(totidx, 255, np.int64)
        pidx_arr = np.zeros(totidx, np.int64)
        val_arr = np.zeros(totidx, np.float32)
        order = np.lexsort((c_tl, c_q, c_r, c_grp))
        so_tid = ((c_grp[order] * 2 + c_r[order]) * 2 + c_q[order]) * ntiles \
            + c_tl[order]
        if len(so_tid):
            starts = np.r_[0, np.flatnonzero(np.diff(so_tid)) + 1]
            run_ids = np.zeros(len(so_tid), np.int64)
            run_ids[starts[1:]] = 1
            run_ids = np.cumsum(run_ids)
            rank = np.arange(len(so_tid)) - starts[run_ids]
            slot = slot_base[c_grp[order], c_r[order], c_q[order],
                             c_tl[order]] + rank
            loc_arr[slot] = c_loc[order]
            pidx_arr[slot] = c_prow[order]
            val_arr[slot] = c_val[order]

        oh_rows = np.zeros((totidx, P), np.float16)
        valid = loc_arr < P
        oh_rows[np.flatnonzero(valid), loc_arr[valid]] = \
            val_arr[valid].astype(np.float16)
        oh = np.ascontiguousarray(
            oh_rows.reshape(totchunks, P, P).transpose(1, 0, 2))
        percore.append({
            "idx": np.ascontiguousarray(_wrap_idx(pidx_arr)),
            "oh": oh,
        })
    return percore, shape


def _prep(inputs):
    x = np.asarray(inputs["x"], np.float32)
    edge_index = np.asarray(inputs["edge_index"], np.int64)
    index = np.asarray(inputs["index"], np.int64)
    W_conv = np.asarray(inputs["W_conv"], np.float32)
    b_conv = np.asarray(inputs["b_conv"], np.float32)
    W1 = np.asarray(inputs["W1"], np.float32)
    b1 = np.asarray(inputs["b1"], np.float32)
    W2 = np.asarray(inputs["W2"], np.float32)
    b2 = np.asarray(inputs["b2"], np.float32)

    n = x.shape[0]
    src = edge_index[0].astype(np.int64)
    dst = edge_index[1].astype(np.int64)
    loops = np.arange(n, dtype=np.int64)
    src = np.concatenate([src, loops])
    dst = np.concatenate([dst, loops])

    deg = np.bincount(dst, minlength=NPAD).astype(np.float32)
    deg[n:] = 1.0
    dinv = np.zeros(NPAD, np.float32)
    dinv[deg > 0] = 1.0 / np.sqrt(deg[deg > 0])

    e_percore, e_shape = _sched_all(
        core=dst // SHARD, tl=(dst % SHARD) // P, loc=dst % P, node=src,
        ntiles=TILES, group_sz=GROUP, ohval=dinv[dst])

    B = index.shape[0]
    PB = B // NC
    PCH = PB // P
    pair_global = np.arange(B, dtype=np.int64)
    pcore = pair_global // PB
    plocal = pair_global % PB
    s_core = np.concatenate([pcore, pcore])
    s_slot = np.concatenate([plocal, PB + plocal])
    s_node = np.concatenate([index[:, 0], index[:, 1]]).astype(np.int64)
    p_percore, p_shape = _sched_all(
        core=s_core, tl=s_slot // P, loc=s_slot % P, node=s_node,
        ntiles=2 * PCH, group_sz=GROUP, ohval=np.ones(len(s_node), np.float32))

    xpad = np.zeros((NPAD, F_IN), np.float32)
    xpad[:n] = x
    xT = xpad.T.astype(np.float16)
    xT_shards = [
        np.ascontiguousarray(
            xT[:, c * SHARD:(c + 1) * SHARD].reshape(2, P, SHARD).transpose(1, 0, 2)
        ) for c in range(NC)
    ]
    dinv_sb = [
        np.ascontiguousarray(
            dinv[c * SHARD:(c + 1) * SHARD].reshape(TILES, P).T
        ) for c in range(NC)
    ]

    consts = {
        "wc": np.ascontiguousarray(
            W_conv.reshape(2, P, H).transpose(1, 0, 2)).astype(np.float16),
        "brow": b_conv.reshape(1, H).astype(np.float16),
        "ones1": np.ones((1, P), np.float16),
        "ident": np.eye(P, dtype=np.float16),
        "w1": W1.astype(np.float16),
        "b1": b1.reshape(16, 1).astype(np.float32),
        "w2": W2.astype(np.float32),
        "b2t": b2.reshape(1, 1).astype(np.float32),
    }
    sched = {"edge": e_shape, "pair": p_shape, "PCH": PCH}
    in_maps = []
    for c in range(NC):
        m = {
            "xt": xT_shards[c],
            "dinv": dinv_sb[c],
            "eidx": e_percore[c]["idx"],
            "eoh": e_percore[c]["oh"],
            "pidx": p_percore[c]["idx"],
            "poh": p_percore[c]["oh"],
        }
        m.update(consts)
        in_maps.append(m)
    return in_maps, sched


def _build(sched, debug=False):
    dt = mybir.dt
    e_sh = sched["edge"]
    p_sh = sched["pair"]
    PCH = sched["PCH"]

    nc = bacc.Bacc("TRN2", target_bir_lowering=False, debug=False,
                   enable_asserts=False, num_devices=NC, num_swdge_queues=4)

    xt_in = nc.dram_tensor("xt", [P, 2, SHARD], dt.float16, kind="ExternalInput")
    dinv_in = nc.dram_tensor("dinv", [P, TILES], dt.float32, kind="ExternalInput")
    eidx_in = nc.dram_tensor("eidx", [P, e_sh["totidx"] // 16], dt.int16,
                             kind="ExternalInput")
    eoh_in = nc.dram_tensor("eoh", [P, e_sh["totchunks"], P], dt.float16,
                            kind="ExternalInput")
    pidx_in = nc.dram_tensor("pidx", [P, p_sh["totidx"] // 16], dt.int16,
                             kind="ExternalInput")
    poh_in = nc.dram_tensor("poh", [P, p_sh["totchunks"], P], dt.float16,
                            kind="ExternalInput")
    wc_in = nc.dram_tensor("wc", [P, 2, H], dt.float16, kind="ExternalInput")
    brow_in = nc.dram_tensor("brow", [1, H], dt.float16, kind="ExternalInput")
    ones1_in = nc.dram_tensor("ones1", [1, P], dt.float16, kind="ExternalInput")
    ident_in = nc.dram_tensor("ident", [P, P], dt.float16, kind="ExternalInput")
    w1_in = nc.dram_tensor("w1", [P, 16], dt.float16, kind="ExternalInput")
    b1_in = nc.dram_tensor("b1", [16, 1], dt.float32, kind="ExternalInput")
    w2_in = nc.dram_tensor("w2", [16, 1], dt.float32, kind="ExternalInput")
    b2_in = nc.dram_tensor("b2t", [1, 1], dt.float32, kind="ExternalInput")
    outp = nc.dram_tensor("out", [PCH * P, 1], dt.float32, kind="ExternalOutput")

    g_shard = [nc.dram_tensor(f"g_shard{r}", [HALF, H], dt.float16)
               for r in (0, 1)]
    g_full = [nc.dram_tensor(f"g_full{r}", [REG, H], dt.float16,
                             addr_space="Shared") for r in (0, 1)]
    e_shard = [nc.dram_tensor(f"e_shard{r}", [HALF, H], dt.float16)
               for r in (0, 1)]
    e_full = [nc.dram_tensor(f"e_full{r}", [REG, H], dt.float16,
                             addr_space="Shared") for r in (0, 1)]

    g_pairs = [g_full[r][:, :].rearrange("(r two) f -> r (two f)", two=2)
               for r in (0, 1)]
    e_pairs = [e_full[r][:, :].rearrange("(r two) f -> r (two f)", two=2)
               for r in (0, 1)]

    def allgather(src_t, dst_t):
        nc.gpsimd.collective_compute(
            "AllGather", mybir.AluOpType.bypass,
            replica_groups=[list(range(NC))],
            ins=[src_t[:, :].opt()],
            outs=[dst_t[:, :].opt()],
        )

    with tile.TileContext(nc) as tc:
        nc.gpsimd.load_library(library_config.mlp)

        with (
            tc.tile_pool(name="const", bufs=1) as cpool,
            tc.tile_pool(name="idxp", bufs=1) as idxp,
        ):
            wc_sb = cpool.tile([P, 2, H], dt.float16)
            nc.sync.dma_start(wc_sb[:], wc_in[:, :, :])
            brow_sb = cpool.tile([1, H], dt.float16)
            nc.sync.dma_start(brow_sb[:], brow_in[:, :])
            ones1_sb = cpool.tile([1, P], dt.float16)
            nc.sync.dma_start(ones1_sb[:], ones1_in[:, :])
            ident = cpool.tile([P, P], dt.float16)
            nc.sync.dma_start(ident[:], ident_in[:, :])
            w1_sb = cpool.tile([P, 16], dt.float16)
            nc.sync.dma_start(w1_sb[:], w1_in[:, :])
            b1_sb = cpool.tile([16, 1], dt.float32)
            nc.sync.dma_start(b1_sb[:], b1_in[:, :])
            w2_sb = cpool.tile([16, 1], dt.float32)
            nc.sync.dma_start(w2_sb[:], w2_in[:, :])
            b2_sb = cpool.tile([1, 1], dt.float32)
            nc.sync.dma_start(b2_sb[:], b2_in[:, :])
            dinv_sb = cpool.tile([P, TILES], dt.float32)
            nc.sync.dma_start(dinv_sb[:], dinv_in[:, :])
            eidx_sb = idxp.tile([P, e_sh["totidx"] // 16], dt.int16)
            nc.sync.dma_start(eidx_sb[:], eidx_in[:, :])
            pidx_sb = idxp.tile([P, p_sh["totidx"] // 16], dt.int16)
            nc.sync.dma_start(pidx_sb[:], pidx_in[:, :])

            # ---------------- phase A: g = (x @ W) * dinv ----------------
            blocks = [(0, 14), (14, 28), (28, 42), (42, 49),
                      (49, 63), (63, 77), (77, 91), (91, 98)]
            with (
                tc.tile_pool(name="xtp", bufs=2) as xtp,
                tc.tile_pool(name="hps", bufs=4, space="PSUM") as hps,
                tc.tile_pool(name="gsb", bufs=1) as gsbp,
            ):
                g_sb = gsbp.tile([P, TILES, H], dt.float16)
                for (t0, t1) in blocks:
                    xt_sb = xtp.tile([P, 2, (t1 - t0) * P], dt.float16, tag="xt")
                    nc.sync.dma_start(xt_sb[:], xt_in[:, :, t0 * P: t1 * P])
                    for t in range(t0, t1):
                        h_ps = hps.tile([P, H], dt.float32)
                        for k in range(2):
                            nc.tensor.matmul(
                                h_ps[:],
                                lhsT=xt_sb[:, k, (t - t0) * P:(t - t0 + 1) * P],
                                rhs=wc_sb[:, k, :],
                                start=(k == 0), stop=(k == 1),
                            )
                        nc.scalar.activation(
                            g_sb[:, t, :], h_ps[:],
                            mybir.ActivationFunctionType.Copy,
                            bias=0.0, scale=dinv_sb[:, t:t + 1])
                    if t1 == HALF_TILES:
                        nc.sync.dma_start(
                            g_shard[0][:, :].rearrange("(t p) f -> p t f", p=P),
                            g_sb[:, :HALF_TILES, :])
                        allgather(g_shard[0], g_full[0])
                nc.sync.dma_start(
                    g_shard[1][:, :].rearrange("(t p) f -> p t f", p=P),
                    g_sb[:, HALF_TILES:, :])
                allgather(g_shard[1], g_full[1])

            # ---------------- phase C: edge aggregation ----------------
            ch_meta = e_sh["chunk_meta"]
            first_chunk = {}
            for ci, (g, r, q, t) in enumerate(ch_meta):
                first_chunk.setdefault(t, ci)

            with (
                tc.tile_pool(name="emsg", bufs=3) as msgp,
                tc.tile_pool(name="eoh", bufs=3) as ohp,
                tc.tile_pool(name="eacc", bufs=2, space="PSUM") as accp,
                tc.tile_pool(name="eemb", bufs=3) as embp,
            ):
                for g in range(NGROUP):
                    t0, t1 = g * GROUP, min((g + 1) * GROUP, TILES)
                    if g == AG2LO_EMIT_GROUP:
                        allgather(e_shard[0], e_full[0])
                    wins = [w for w in e_sh["win_meta"] if w[0] == g]
                    ohs, msgs = {}, {}
                    for (gg, r, q, c0, nch) in wins:
                        if nch == 0:
                            continue
                        oh = ohp.tile([P, nch, P], dt.float16,
                                      tag=f"eoh{r}{q}", name=f"eoh_g{g}r{r}q{q}")
                        nc.sync.dma_start(oh[:], eoh_in[:, c0:c0 + nch, :])
                        ohs[(r, q)] = oh
                    for (gg, r, q, c0, nch) in wins:
                        if nch == 0:
                            continue
                        msg = msgp.tile([P, nch, P], dt.float16,
                                        tag=f"emsg{r}{q}",
                                        name=f"emsg_g{g}r{r}q{q}")
                        nc.gpsimd.dma_gather(
                            msg[:], g_pairs[r],
                            eidx_sb[:, c0 * P // 16:(c0 + nch) * P // 16],
                            nch * P, nch * P, P,
                            single_packet=False, queue_num=0)
                        msgs[(r, q)] = msg
                    acc_tiles = {}
                    for (gg, r, q, c0, nch) in wins:
                        if nch == 0:
                            continue
                        oh, msg = ohs[(r, q)], msgs[(r, q)]
                        for ci in range(c0, c0 + nch):
                            _, rr, qq, t = ch_meta[ci]
                            if t not in acc_tiles:
                                acc_tiles[t] = accp.tile(
                                    [P, H], dt.float32, tag=f"eacc{t % GROUP}",
                                    name=f"eacc_t{t}")
                            nc.tensor.matmul(
                                acc_tiles[t][:],
                                lhsT=oh[:, ci - c0, :],
                                rhs=msg[:, ci - c0, qq * H:(qq + 1) * H],
                                start=(ci == first_chunk[t]),
                                stop=False,
                            )
                    emb_stage = embp.tile([P, t1 - t0, H], dt.float16,
                                          tag="emb", name=f"emb_g{g}")
                    for t in range(t0, t1):
                        a = acc_tiles.pop(t)
                        nc.tensor.matmul(
                            a[:], lhsT=ones1_sb[:], rhs=brow_sb[:],
                            start=False, stop=True)
                        nc.scalar.activation(
                            emb_stage[:, t - t0, :], a[:],
                            mybir.ActivationFunctionType.Lrelu,
                            bias=0.0, scale=1.0, alpha=NEG)
                    # flush stage (split at the lo/hi region boundary)
                    spans = []
                    if t0 < HALF_TILES:
                        spans.append((t0, min(t1, HALF_TILES), 0))
                    if t1 > HALF_TILES:
                        spans.append((max(t0, HALF_TILES), t1, 1))
                    for (lo, hi, reg) in spans:
                        base = lo - (0 if reg == 0 else HALF_TILES)
                        nc.sync.dma_start(
                            e_shard[reg][base * P:(base + hi - lo) * P, :]
                            .rearrange("(t p) f -> p t f", p=P),
                            emb_stage[:, lo - t0:hi - t0, :])

                allgather(e_shard[1], e_full[1])

            # ---------------- phase D: pair gather + MLP ----------------
            pch_meta = p_sh["chunk_meta"]
            pfirst = {}
            plast = {}
            for ci, (g, r, q, t) in enumerate(pch_meta):
                pfirst.setdefault(t, ci)
                plast[t] = ci

            with (
                tc.tile_pool(name="pmsg", bufs=2) as pmsgp,
                tc.tile_pool(name="poh", bufs=2) as pohp,
                tc.tile_pool(name="pacc", bufs=1, space="PSUM") as paccp,
                tc.tile_pool(name="pxs", bufs=1) as pxsp,
                tc.tile_pool(name="ptps", bufs=2, space="PSUM") as ptps,
                tc.tile_pool(name="pzps", bufs=1, space="PSUM") as pzps,
                tc.tile_pool(name="pops", bufs=1, space="PSUM") as pops,
                tc.tile_pool(name="psb", bufs=4) as psbp,
            ):
                xs_sb = pxsp.tile([P, 2 * PCH, H], dt.float16)
                pgroups = (2 * PCH + GROUP - 1) // GROUP
                for g in range(pgroups):
                    t0, t1 = g * GROUP, min((g + 1) * GROUP, 2 * PCH)
                    wins = [w for w in p_sh["win_meta"] if w[0] == g]
                    ohs, msgs = {}, {}
                    for (gg, r, q, c0, nch) in wins:
                        if nch == 0:
                            continue
                        oh = pohp.tile([P, nch, P], dt.float16,
                                       tag=f"poh{r}{q}", name=f"poh_g{g}r{r}q{q}")
                        nc.sync.dma_start(oh[:], poh_in[:, c0:c0 + nch, :])
                        ohs[(r, q)] = oh
                    for (gg, r, q, c0, nch) in wins:
                        if nch == 0:
                            continue
                        msg = pmsgp.tile([P, nch, P], dt.float16,
                                         tag=f"pmsg{r}{q}",
                                         name=f"pmsg_g{g}r{r}q{q}")
                        nc.gpsimd.dma_gather(
                            msg[:], e_pairs[r],
                            pidx_sb[:, c0 * P // 16:(c0 + nch) * P // 16],
                            nch * P, nch * P, P,
                            single_packet=False, queue_num=0)
                        msgs[(r, q)] = msg
                    acc_tiles = {}
                    for (gg, r, q, c0, nch) in wins:
                        if nch == 0:
                            continue
                        oh, msg = ohs[(r, q)], msgs[(r, q)]
                        for ci in range(c0, c0 + nch):
                            _, rr, qq, t = pch_meta[ci]
                            if t not in acc_tiles:
                                acc_tiles[t] = paccp.tile(
                                    [P, H], dt.float32, tag=f"pacc{t % GROUP}",
                                    name=f"pacc_t{t}")
                            nc.tensor.matmul(
                                acc_tiles[t][:],
                                lhsT=oh[:, ci - c0, :],
                                rhs=msg[:, ci - c0, qq * H:(qq + 1) * H],
                                start=(ci == pfirst[t]),
                                stop=(ci == plast[t]),
                            )
                    for t in range(t0, t1):
                        nc.scalar.activation(
                            xs_sb[:, t, :], acc_tiles.pop(t)[:],
                            mybir.ActivationFunctionType.Copy,
                            bias=0.0, scale=1.0)

                for k in range(PCH):
                    xt_ps = ptps.tile([P, P], dt.float16)
                    nc.tensor.transpose(xt_ps[0:H, :], xs_sb[:, k, :], ident[:])
                    nc.tensor.transpose(xt_ps[H:P, :], xs_sb[:, PCH + k, :],
                                        ident[:])
                    xijt = psbp.tile([P, P], dt.float16, tag="xijt")
                    nc.scalar.activation(
                        xijt[:], xt_ps[:], mybir.ActivationFunctionType.Copy,
                        bias=0.0, scale=1.0)
                    z_ps = pzps.tile([16, P], dt.float32)
                    nc.tensor.matmul(z_ps[:], lhsT=w1_sb[:], rhs=xijt[:],
                                     start=True, stop=True)
                    z2 = psbp.tile([16, P], dt.float32, tag="z2")
                    nc.scalar.activation(
                        z2[:], z_ps[:], mybir.ActivationFunctionType.Lrelu,
                        bias=b1_sb[:, 0:1], scale=1.0, alpha=NEG)
                    o_ps = pops.tile([1, P], dt.float32)
                    nc.tensor.matmul(o_ps[:], lhsT=w2_sb[:], rhs=z2[:],
                                     start=True, stop=True)
                    osb = psbp.tile([1, P], dt.float32, tag="osb")
                    nc.scalar.activation(
                        osb[:], o_ps[:], mybir.ActivationFunctionType.Sigmoid,
                        bias=b2_sb[:, 0:1], scale=1.0)
                    nc.sync.dma_start(
                        outp[k * P:(k + 1) * P, :].rearrange("r one -> one r"),
                        osb[0:1, :])

    # align each gather's SWDGE queue with its Tile-assigned DMA lane so
    # semaphore<->queue locking stays consistent (4-way parallel desc gen)
    for blk in nc.m.functions[0].blocks:
        for inst in blk.instructions:
            if isinstance(inst, mybir.InstDMAGatherAnt):
                si = inst.sync_info
                for u in (si.on_update if si else []):
                    mm = re.match(r"DMASW(\d+)_", u.ant_name or "")
                    if mm:
                        inst.queue_num = int(mm.group(1)) % 4
                        break

    nc.compile()
    return nc


def kernel(**inputs) -> np.ndarray:
    in_maps, sched = _prep(inputs)
    nc = _build(sched)
    res = run_bass_kernel_spmd(nc, in_maps, list(range(NC)))
    out = np.concatenate([res.results[c]["out"] for c in range(NC)], axis=0)
    return out.astype(np.float32)


# revision 11
# speedup vs baseline: 1.1247x; 1.0471x over previous
"""GCN message-passing kernel for 8 Trainium2 NeuronCores (Bass/Tile).

Computes (matching the jax reference):
    h = x @ W_conv                      [N, H]
    node_embed = leaky_relu(D^-1/2 (A+I) D^-1/2 h + b_conv)
    out = sigmoid(leaky(cat(e[i], e[j]) @ W1 + b1) @ W2 + b2)

Distribution: nodes dst-sharded over 8 cores. Scaled features g = dinv*h are
AllGathered in two halves (each shard split lo/hi) so edge windows sourcing
the lo half start while the hi half is in flight. Per-edge source rows are
fetched with dma_gather (pair-packed fp16 rows) spread across the 4 SWDGE
queues; scatter-add onto destination tiles runs on the TensorEngine via
one-hot matmuls whose one-hot operands are HOST-precomputed (dinv[dst]
folded in) and streamed from HBM — the DVE is kept entirely idle during
gather phases because GpSimd SWDGE and DVE perf-mode ops take an exclusive
full-instruction lock on a shared SBUF port pair. Per-tile bias lands via a
rank-1 matmul; leaky_relu runs on the ACT engine (own ports). The e
AllGather is likewise split and overlapped with the tail of phase C.
"""

import re

import numpy as np

import concourse.bass as bass
import concourse.bacc as bacc
import concourse.mybir as mybir
import concourse.tile as tile
from concourse import library_config
from concourse.bass_utils import run_bass_kernel_spmd

NC = 8
N_NODES = 100000
F_IN = 256
H = 64
NEG = 0.01

P = 128                    # partitions / tile height
TILES = 98                 # node tiles per core
SHARD = TILES * P          # 12544 nodes per core
NPAD = NC * SHARD          # 100352
HALF_TILES = 49
HALF = HALF_TILES * P      # 6272 nodes = lo region of each shard
REG = NC * HALF            # 50176 nodes per region
RPAIR = REG // 2           # 25088 pair rows per region (int16-addressable)
GROUP = 4                  # dst node tiles per edge gather group
NGROUP = (TILES + GROUP - 1) // GROUP
AG2LO_EMIT_GROUP = 15      # emit e-lo AllGather before this C group


def _node_addr(n):
    """node id -> (region, pair_row_in_region, parity)."""
    c = n // SHARD
    l = n % SHARD
    r = (l >= HALF).astype(np.int64)
    pos = c * HALF + (l - r * HALF)
    return r, pos >> 1, pos & 1


def _wrap_idx(idx):
    """int array [W] (W % 16 == 0) -> [128, W//16] int16 wrapped/replicated."""
    w = idx.reshape(-1, 16).T.astype(np.int16)
    return np.tile(w, (8, 1))


def _sched_all(core, tl, loc, node, ntiles, group_sz, ohval):
    """Single-pass schedule: bins (g, r, q, t), windows (g, r, q), chunk
    order g -> r -> q -> t -> k. K maxes over cores (SPMD-uniform shapes).

    ohval[item] = value placed in the one-hot at the item's (slot, loc).
    Returns (percore list of {idx, oh}, shape dict).
    """
    region, prow, parity = _node_addr(node)
    ngroups = (ntiles + group_sz - 1) // group_sz
    grp = tl // group_sz

    tid = (((core * ngroups + grp) * 2 + region) * 2 + parity) * ntiles + tl
    n_bins = NC * ngroups * 2 * 2 * ntiles
    cnt = np.bincount(tid, minlength=n_bins).reshape(NC, ngroups, 2, 2, ntiles)
    K = (cnt + P - 1) // P
    K = K.max(axis=0)                          # [ngroups, 2, 2, ntiles]
    for t in range(ntiles):
        g = t // group_sz
        if K[g, :, :, t].sum() == 0:
            K[g, 0, 0, t] = 1                  # keep acc defined

    chunk_meta = []                            # (g, r, q, t)
    win_meta = []                              # (g, r, q, c0, nch, w)
    slot_base = np.zeros((ngroups, 2, 2, ntiles), np.int64)
    for g in range(ngroups):
        t0, t1 = g * group_sz, min((g + 1) * group_sz, ntiles)
        for r in (0, 1):
            for q in (0, 1):
                c0 = len(chunk_meta)
                for t in range(t0, t1):
                    slot_base[g, r, q, t] = len(chunk_meta) * P
                    for _ in range(K[g, r, q, t]):
                        chunk_meta.append((g, r, q, t))
                win_meta.append((g, r, q, c0, len(chunk_meta) - c0,
                                 len(win_meta)))
    totchunks = len(chunk_meta)
    totidx = totchunks * P
    wmax = max(nch for (_, _, _, _, nch, _) in win_meta)
    nwin = len(win_meta)
    tab_totidx = totidx

    shape = {
        "chunk_meta": chunk_meta,
        "win_meta": win_meta,
        "totchunks": totchunks,
        "totidx": totidx,
        "wmax": wmax,
        "nwin": nwin,
        "tab_totidx": tab_totidx,
        "ntiles": ntiles,
        "ngroups": ngroups,
        "group_sz": group_sz,
    }

    percore = []
    for c in range(NC):
        msk = core == c
        c_tl, c_loc = tl[msk], loc[msk]
        c_grp, c_r, c_q = grp[msk], region[msk], parity[msk]
        c_prow, c_val = prow[msk], ohval[msk]
        loc_arr = np.full(totidx, 255, np.int64)
        pidx_arr = np.zeros(totidx, np.int64)
        val_arr = np.zeros(totidx, np.float32)
        order = np.lexsort((c_tl, c_q, c_r, c_grp))
        so_tid = ((c_grp[order] * 2 + c_r[order]) * 2 + c_q[order]) * ntiles \
            + c_tl[order]
        if len(so_tid):
            starts = np.r_[0, np.flatnonzero(np.diff(so_tid)) + 1]
            run_ids = np.zeros(len(so_tid), np.int64)
            run_ids[starts[1:]] = 1
            run_ids = np.cumsum(run_ids)
            rank = np.arange(len(so_tid)) - starts[run_ids]
            slot = slot_base[c_grp[order], c_r[order], c_q[order],
                             c_tl[order]] + rank
            loc_arr[slot] = c_loc[order]
            pidx_arr[slot] = c_prow[order]
            val_arr[slot] = c_val[order]

        oh_rows = np.zeros((totidx, P), np.float16)
        valid = loc_arr < P
        oh_rows[np.flatnonzero(valid), loc_arr[valid]] = \
            val_arr[valid].astype(np.float16)
        oh = np.ascontiguousarray(
            oh_rows.reshape(totchunks, P, P).transpose(1, 0, 2))
        percore.append({
            "idx": np.ascontiguousarray(_wrap_idx(pidx_arr)),
            "oh": oh,
        })
    return percore, shape


def _prep(inputs):
    x = np.asarray(inputs["x"], np.float32)
    edge_index = np.asarray(inputs["edge_index"], np.int64)
    index = np.asarray(inputs["index"], np.int64)
    W_conv = np.asarray(inputs["W_conv"], np.float32)
    b_conv = np.asarray(inputs["b_conv"], np.float32)
    W1 = np.asarray(inputs["W1"], np.float32)
    b1 = np.asarray(inputs["b1"], np.float32)
    W2 = np.asarray(inputs["W2"], np.float32)
    b2 = np.asarray(inputs["b2"], np.float32)

    n = x.shape[0]
    src = edge_index[0].astype(np.int64)
    dst = edge_index[1].astype(np.int64)
    loops = np.arange(n, dtype=np.int64)
    src = np.concatenate([src, loops])
    dst = np.concatenate([dst, loops])

    deg = np.bincount(dst, minlength=NPAD).astype(np.float32)
    deg[n:] = 1.0
    dinv = np.zeros(NPAD, np.float32)
    dinv[deg > 0] = 1.0 / np.sqrt(deg[deg > 0])

    e_percore, e_shape = _sched_all(
        core=dst // SHARD, tl=(dst % SHARD) // P, loc=dst % P, node=src,
        ntiles=TILES, group_sz=GROUP, ohval=dinv[dst])

    B = index.shape[0]
    PB = B // NC
    PCH = PB // P
    pair_global = np.arange(B, dtype=np.int64)
    pcore = pair_global // PB
    plocal = pair_global % PB
    s_core = np.concatenate([pcore, pcore])
    s_slot = np.concatenate([plocal, PB + plocal])
    s_node = np.concatenate([index[:, 0], index[:, 1]]).astype(np.int64)
    p_percore, p_shape = _sched_all(
        core=s_core, tl=s_slot // P, loc=s_slot % P, node=s_node,
        ntiles=2 * PCH, group_sz=GROUP, ohval=np.ones(len(s_node), np.float32))

    xpad = np.zeros((NPAD, F_IN), np.float32)
    xpad[:n] = x
    xT = xpad.T.astype(np.float16)
    xT_shards = [
        np.ascontiguousarray(
            xT[:, c * SHARD:(c + 1) * SHARD].reshape(2, P, SHARD).transpose(1, 0, 2)
        ) for c in range(NC)
    ]
    dinv_sb = [
        np.ascontiguousarray(
            dinv[c * SHARD:(c + 1) * SHARD].reshape(TILES, P).T
        ) for c in range(NC)
    ]

    consts = {
        "wc": np.ascontiguousarray(
            W_conv.reshape(2, P, H).transpose(1, 0, 2)).astype(np.float16),
        "brow": b_conv.reshape(1, H).astype(np.float16),
        "ones1": np.ones((1, P), np.float16),
        "ident": np.eye(P, dtype=np.float16),
        "w1": W1.astype(np.float16),
        "b1": b1.reshape(16, 1).astype(np.float32),
        "w2": W2.astype(np.float32),
        "b2t": b2.reshape(1, 1).astype(np.float32),
    }
    sched = {"edge": e_shape, "pair": p_shape, "PCH": PCH}
    in_maps = []
    for c in range(NC):
        m = {
            "xt": xT_shards[c],
            "dinv": dinv_sb[c],
            "eidx": e_percore[c]["idx"],
            "eoh": e_percore[c]["oh"],
            "pidx": p_percore[c]["idx"],
            "poh": p_percore[c]["oh"],
        }
        m.update(consts)
        in_maps.append(m)
    return in_maps, sched


def _build(sched, debug=False):
    dt = mybir.dt
    e_sh = sched["edge"]
    p_sh = sched["pair"]
    PCH = sched["PCH"]

    nc = bacc.Bacc("TRN2", target_bir_lowering=False, debug=False,
                   enable_asserts=False, num_devices=NC, num_swdge_queues=4)

    xt_in = nc.dram_tensor("xt", [P, 2, SHARD], dt.float16, kind="ExternalInput")
    dinv_in = nc.dram_tensor("dinv", [P, TILES], dt.float32, kind="ExternalInput")
    eidx_in = nc.dram_tensor("eidx", [P, e_sh["tab_totidx"] // 16], dt.int16,
                             kind="ExternalInput")
    eoh_in = nc.dram_tensor("eoh", [P, e_sh["totchunks"], P], dt.float16,
                            kind="ExternalInput")
    pidx_in = nc.dram_tensor("pidx", [P, p_sh["tab_totidx"] // 16], dt.int16,
                             kind="ExternalInput")
    poh_in = nc.dram_tensor("poh", [P, p_sh["totchunks"], P], dt.float16,
                            kind="ExternalInput")
    wc_in = nc.dram_tensor("wc", [P, 2, H], dt.float16, kind="ExternalInput")
    brow_in = nc.dram_tensor("brow", [1, H], dt.float16, kind="ExternalInput")
    ones1_in = nc.dram_tensor("ones1", [1, P], dt.float16, kind="ExternalInput")
    ident_in = nc.dram_tensor("ident", [P, P], dt.float16, kind="ExternalInput")
    w1_in = nc.dram_tensor("w1", [P, 16], dt.float16, kind="ExternalInput")
    b1_in = nc.dram_tensor("b1", [16, 1], dt.float32, kind="ExternalInput")
    w2_in = nc.dram_tensor("w2", [16, 1], dt.float32, kind="ExternalInput")
    b2_in = nc.dram_tensor("b2t", [1, 1], dt.float32, kind="ExternalInput")
    outp = nc.dram_tensor("out", [PCH * P, 1], dt.float32, kind="ExternalOutput")

    g_shard = [nc.dram_tensor(f"g_shard{r}", [HALF, H], dt.float16)
               for r in (0, 1)]
    g_full = [nc.dram_tensor(f"g_full{r}", [REG, H], dt.float16,
                             addr_space="Shared") for r in (0, 1)]
    e_shard = [nc.dram_tensor(f"e_shard{r}", [HALF, H], dt.float16)
               for r in (0, 1)]
    e_full = [nc.dram_tensor(f"e_full{r}", [REG, H], dt.float16,
                             addr_space="Shared") for r in (0, 1)]

    g_pairs = [g_full[r][:, :].rearrange("(r two) f -> r (two f)", two=2)
               for r in (0, 1)]
    e_pairs = [e_full[r][:, :].rearrange("(r two) f -> r (two f)", two=2)
               for r in (0, 1)]

    def allgather(src_t, dst_t):
        nc.gpsimd.collective_compute(
            "AllGather", mybir.AluOpType.bypass,
            replica_groups=[list(range(NC))],
            ins=[src_t[:, :].opt()],
            outs=[dst_t[:, :].opt()],
        )

    with tile.TileContext(nc) as tc:
        nc.gpsimd.load_library(library_config.mlp)

        with (
            tc.tile_pool(name="const", bufs=1) as cpool,
            tc.tile_pool(name="idxp", bufs=1) as idxp,
        ):
            wc_sb = cpool.tile([P, 2, H], dt.float16)
            nc.sync.dma_start(wc_sb[:], wc_in[:, :, :])
            brow_sb = cpool.tile([1, H], dt.float16)
            nc.sync.dma_start(brow_sb[:], brow_in[:, :])
            ones1_sb = cpool.tile([1, P], dt.float16)
            nc.sync.dma_start(ones1_sb[:], ones1_in[:, :])
            ident = cpool.tile([P, P], dt.float16)
            nc.sync.dma_start(ident[:], ident_in[:, :])
            w1_sb = cpool.tile([P, 16], dt.float16)
            nc.sync.dma_start(w1_sb[:], w1_in[:, :])
            b1_sb = cpool.tile([16, 1], dt.float32)
            nc.sync.dma_start(b1_sb[:], b1_in[:, :])
            w2_sb = cpool.tile([16, 1], dt.float32)
            nc.sync.dma_start(w2_sb[:], w2_in[:, :])
            b2_sb = cpool.tile([1, 1], dt.float32)
            nc.sync.dma_start(b2_sb[:], b2_in[:, :])
            dinv_sb = cpool.tile([P, TILES], dt.float32)
            nc.sync.dma_start(dinv_sb[:], dinv_in[:, :])
            eidx_sb = idxp.tile([P, e_sh["tab_totidx"] // 16], dt.int16)
            nc.sync.dma_start(eidx_sb[:], eidx_in[:, :])
            pidx_sb = idxp.tile([P, p_sh["tab_totidx"] // 16], dt.int16)
            nc.sync.dma_start(pidx_sb[:], pidx_in[:, :])

            # one shared register per distinct gather size: a fresh immediate
            # per gather would serialize the Pool stream on a register WAR
            # hazard against the in-flight gather's num_idxs register read
            sizes = sorted({nch * P
                            for sh in (e_sh, p_sh)
                            for (_, _, _, _, nch, _) in sh["win_meta"]
                            if nch > 0})
            nidx_regs = {v: nc.gpsimd.to_reg(v) for v in sizes}

            # ---------------- phase A: g = (x @ W) * dinv ----------------
            blocks = [(0, 14), (14, 28), (28, 42), (42, 49),
                      (49, 63), (63, 77), (77, 91), (91, 98)]
            with (
                tc.tile_pool(name="xtp", bufs=2) as xtp,
                tc.tile_pool(name="hps", bufs=4, space="PSUM") as hps,
                tc.tile_pool(name="gsb", bufs=1) as gsbp,
            ):
                g_sb = gsbp.tile([P, TILES, H], dt.float16)
                for (t0, t1) in blocks:
                    xt_sb = xtp.tile([P, 2, (t1 - t0) * P], dt.float16, tag="xt")
                    nc.sync.dma_start(xt_sb[:], xt_in[:, :, t0 * P: t1 * P])
                    for t in range(t0, t1):
                        h_ps = hps.tile([P, H], dt.float32)
                        for k in range(2):
                            nc.tensor.matmul(
                                h_ps[:],
                                lhsT=xt_sb[:, k, (t - t0) * P:(t - t0 + 1) * P],
                                rhs=wc_sb[:, k, :],
                                start=(k == 0), stop=(k == 1),
                            )
                        nc.scalar.activation(
                            g_sb[:, t, :], h_ps[:],
                            mybir.ActivationFunctionType.Copy,
                            bias=0.0, scale=dinv_sb[:, t:t + 1])
                    if t1 == HALF_TILES:
                        nc.sync.dma_start(
                            g_shard[0][:, :].rearrange("(t p) f -> p t f", p=P),
                            g_sb[:, :HALF_TILES, :])
                        allgather(g_shard[0], g_full[0])
                nc.sync.dma_start(
                    g_shard[1][:, :].rearrange("(t p) f -> p t f", p=P),
                    g_sb[:, HALF_TILES:, :])
                allgather(g_shard[1], g_full[1])

            # ---------------- phase C: edge aggregation ----------------
            ch_meta = e_sh["chunk_meta"]
            first_chunk = {}
            for ci, (g, r, q, t) in enumerate(ch_meta):
                first_chunk.setdefault(t, ci)

            with (
                tc.tile_pool(name="emsg", bufs=3) as msgp,
                tc.tile_pool(name="eoh", bufs=3) as ohp,
                tc.tile_pool(name="eacc", bufs=2, space="PSUM") as accp,
                tc.tile_pool(name="eemb", bufs=3) as embp,
            ):
                for g in range(NGROUP):
                    t0, t1 = g * GROUP, min((g + 1) * GROUP, TILES)
                    if g == AG2LO_EMIT_GROUP:
                        allgather(e_shard[0], e_full[0])
                    wins = [w for w in e_sh["win_meta"] if w[0] == g]
                    WM = e_sh["wmax"]
                    ohs, msgs = {}, {}
                    for (gg, r, q, c0, nch, w) in wins:
                        if nch == 0:
                            continue
                        oh = ohp.tile([P, WM, P], dt.float16,
                                      tag=f"eoh{r}{q}", name=f"eoh_g{g}r{r}q{q}")
                        nc.sync.dma_start(oh[:, :nch, :], eoh_in[:, c0:c0 + nch, :])
                        ohs[(r, q)] = oh
                    for (gg, r, q, c0, nch, w) in wins:
                        if nch == 0:
                            continue
                        msg = msgp.tile([P, WM, P], dt.float16,
                                        tag=f"emsg{r}{q}",
                                        name=f"emsg_g{g}r{r}q{q}")
                        nc.gpsimd.dma_gather(
                            msg[:, :nch, :], g_pairs[r],
                            eidx_sb[:, c0 * P // 16:(c0 + nch) * P // 16],
                            nch * P, nidx_regs[nch * P], P,
                            single_packet=False, queue_num=0)
                        msgs[(r, q)] = msg
                    acc_tiles = {}
                    for (gg, r, q, c0, nch, w) in wins:
                        if nch == 0:
                            continue
                        oh, msg = ohs[(r, q)], msgs[(r, q)]
                        for ci in range(c0, c0 + nch):
                            _, rr, qq, t = ch_meta[ci]
                            if t not in acc_tiles:
                                acc_tiles[t] = accp.tile(
                                    [P, H], dt.float32, tag=f"eacc{t % GROUP}",
                                    name=f"eacc_t{t}")
                            nc.tensor.matmul(
                                acc_tiles[t][:],
                                lhsT=oh[:, ci - c0, :],
                                rhs=msg[:, ci - c0, qq * H:(qq + 1) * H],
                                start=(ci == first_chunk[t]),
                                stop=False,
                            )
                    emb_stage = embp.tile([P, t1 - t0, H], dt.float16,
                                          tag="emb", name=f"emb_g{g}")
                    for t in range(t0, t1):
                        a = acc_tiles.pop(t)
                        nc.tensor.matmul(
                            a[:], lhsT=ones1_sb[:], rhs=brow_sb[:],
                            start=False, stop=True)
                        nc.scalar.activation(
                            emb_stage[:, t - t0, :], a[:],
                            mybir.ActivationFunctionType.Lrelu,
                            bias=0.0, scale=1.0, alpha=NEG)
                    # flush stage (split at the lo/hi region boundary)
                    spans = []
                    if t0 < HALF_TILES:
                        spans.append((t0, min(t1, HALF_TILES), 0))
                    if t1 > HALF_TILES:
                        spans.append((max(t0, HALF_TILES), t1, 1))
                    for (lo, hi, reg) in spans:
                        base = lo - (0 if reg == 0 else HALF_TILES)
                        nc.sync.dma_start(
                            e_shard[reg][base * P:(base + hi - lo) * P, :]
                            .rearrange("(t p) f -> p t f", p=P),
                            emb_stage[:, lo - t0:hi - t0, :])

                allgather(e_shard[1], e_full[1])

            # ---------------- phase D: pair gather + MLP ----------------
            pch_meta = p_sh["chunk_meta"]
            pfirst = {}
            plast = {}
            for ci, (g, r, q, t) in enumerate(pch_meta):
                pfirst.setdefault(t, ci)
                plast[t] = ci

            with (
                tc.tile_pool(name="pmsg", bufs=2) as pmsgp,
                tc.tile_pool(name="poh", bufs=2) as pohp,
                tc.tile_pool(name="pacc", bufs=1, space="PSUM") as paccp,
                tc.tile_pool(name="pxs", bufs=1) as pxsp,
                tc.tile_pool(name="ptps", bufs=2, space="PSUM") as ptps,
                tc.tile_pool(name="pzps", bufs=1, space="PSUM") as pzps,
                tc.tile_pool(name="pops", bufs=1, space="PSUM") as pops,
                tc.tile_pool(name="psb", bufs=4) as psbp,
            ):
                xs_sb = pxsp.tile([P, 2 * PCH, H], dt.float16)
                pgroups = (2 * PCH + GROUP - 1) // GROUP
                for g in range(pgroups):
                    t0, t1 = g * GROUP, min((g + 1) * GROUP, 2 * PCH)
                    wins = [w for w in p_sh["win_meta"] if w[0] == g]
                    WMP = p_sh["wmax"]
                    ohs, msgs = {}, {}
                    for (gg, r, q, c0, nch, w) in wins:
                        if nch == 0:
                            continue
                        oh = pohp.tile([P, WMP, P], dt.float16,
                                       tag=f"poh{r}{q}", name=f"poh_g{g}r{r}q{q}")
                        nc.sync.dma_start(oh[:, :nch, :], poh_in[:, c0:c0 + nch, :])
                        ohs[(r, q)] = oh
                    for (gg, r, q, c0, nch, w) in wins:
                        if nch == 0:
                            continue
                        msg = pmsgp.tile([P, WMP, P], dt.float16,
                                         tag=f"pmsg{r}{q}",
                                         name=f"pmsg_g{g}r{r}q{q}")
                        nc.gpsimd.dma_gather(
                            msg[:, :nch, :], e_pairs[r],
                            pidx_sb[:, c0 * P // 16:(c0 + nch) * P // 16],
                            nch * P, nidx_regs[nch * P], P,
                            single_packet=False, queue_num=0)
                        msgs[(r, q)] = msg
                    acc_tiles = {}
                    for (gg, r, q, c0, nch, w) in wins:
                        if nch == 0:
                            continue
                        oh, msg = ohs[(r, q)], msgs[(r, q)]
                        for ci in range(c0, c0 + nch):
                            _, rr, qq, t = pch_meta[ci]
                            if t not in acc_tiles:
                                acc_tiles[t] = paccp.tile(
                                    [P, H], dt.float32, tag=f"pacc{t % GROUP}",
                                    name=f"pacc_t{t}")
                            nc.tensor.matmul(
                                acc_tiles[t][:],
                                lhsT=oh[:, ci - c0, :],
                                rhs=msg[:, ci - c0, qq * H:(qq + 1) * H],
                                start=(ci == pfirst[t]),
                                stop=(ci == plast[t]),
                            )
                    for t in range(t0, t1):
                        nc.scalar.activation(
                            xs_sb[:, t, :], acc_tiles.pop(t)[:],
                            mybir.ActivationFunctionType.Copy,
                            bias=0.0, scale=1.0)

                z2s = []
                for k in range(PCH):
                    xt_ps = ptps.tile([P, P], dt.float16)
                    nc.tensor.transpose(xt_ps[0:H, :], xs_sb[:, k, :], ident[:])
                    nc.tensor.transpose(xt_ps[H:P, :], xs_sb[:, PCH + k, :],
                                        ident[:])
                    xijt = psbp.tile([P, P], dt.float16, tag="xijt")
                    nc.scalar.activation(
                        xijt[:], xt_ps[:], mybir.ActivationFunctionType.Copy,
                        bias=0.0, scale=1.0)
                    z_ps = pzps.tile([16, P], dt.float32)
                    nc.tensor.matmul(z_ps[:], lhsT=w1_sb[:], rhs=xijt[:],
                                     start=True, stop=True)
                    z2 = psbp.tile([16, P], dt.float32, tag=f"z2_{k}")
                    nc.scalar.activation(
                        z2[:], z_ps[:], mybir.ActivationFunctionType.Lrelu,
                        bias=b1_sb[:, 0:1], scale=1.0, alpha=NEG)
                    z2s.append(z2)
                for k in range(PCH):
                    o_ps = pops.tile([1, P], dt.float32)
                    nc.tensor.matmul(o_ps[:], lhsT=w2_sb[:], rhs=z2s[k][:],
                                     start=True, stop=True)
                    osb = psbp.tile([1, P], dt.float32, tag="osb")
                    nc.scalar.activation(
                        osb[:], o_ps[:], mybir.ActivationFunctionType.Sigmoid,
                        bias=b2_sb[:, 0:1], scale=1.0)
                    nc.sync.dma_start(
                        outp[k * P:(k + 1) * P, :].rearrange("r one -> one r"),
                        osb[0:1, :])

    # align each gather's SWDGE queue with its Tile-assigned DMA lane so
    # semaphore<->queue locking stays consistent (4-way parallel desc gen)
    for blk in nc.m.functions[0].blocks:
        for inst in blk.instructions:
            if isinstance(inst, mybir.InstDMAGatherAnt):
                si = inst.sync_info
                for u in (si.on_update if si else []):
                    mm = re.match(r"DMASW(\d+)_", u.ant_name or "")
                    if mm:
                        inst.queue_num = int(mm.group(1)) % 4
                        break

    nc.compile()
    return nc


def kernel(**inputs) -> np.ndarray:
    in_maps, sched = _prep(inputs)
    nc = _build(sched)
    res = run_bass_kernel_spmd(nc, in_maps, list(range(NC)))
    out = np.concatenate([res.results[c]["out"] for c in range(NC)], axis=0)
    return out.astype(np.float32)


# revision 12
# speedup vs baseline: 1.2356x; 1.0987x over previous
"""GCN message-passing kernel for 8 Trainium2 NeuronCores (Bass/Tile).

Computes (matching the jax reference):
    h = x @ W_conv                      [N, H]
    node_embed = leaky_relu(D^-1/2 (A+I) D^-1/2 h + b_conv)
    out = sigmoid(leaky(cat(e[i], e[j]) @ W1 + b1) @ W2 + b2)

Distribution: nodes are sharded over the 8 cores (dst-sharded aggregation
with an AllGather of the scaled features g = dinv * h). Edges are
partitioned by destination shard and scatter-added on the TensorEngine
via per-destination-tile one-hot matmuls; per-edge source rows are
fetched with bulk dma_gather (pair-packed fp16 rows, 4 SWDGE queues).
The pair-MLP head reuses the same gather + one-hot-permute machinery to
collect node embeddings in pair order.
"""

import re

import numpy as np

import concourse.bass as bass
import concourse.bacc as bacc
import concourse.mybir as mybir
import concourse.tile as tile
from concourse import library_config
from concourse.bass_utils import run_bass_kernel_spmd

NC = 8
N_NODES = 100000
F_IN = 256
H = 64
NEG = 0.01

P = 128                    # partitions / tile height
TILES = 98                 # node tiles per core
SHARD = TILES * P          # 12544 nodes per core
NPAD = NC * SHARD          # 100352
GROUP = 4                  # node tiles per edge gather group
NBUCKET = 4                # (src range) x (src parity)
PGROUP = 4                 # pair slot-tiles per gather group


def half_range():
    return NPAD // 4       # 25088: int16-addressable pair rows per range


def _wrap_idx_window(idx):
    """int array [W] (W % 16 == 0) -> [128, W//16] int16 wrapped/replicated."""
    w = idx.reshape(-1, 16).T.astype(np.int16)
    return np.tile(w, (8, 1))


def _scatter_sched(core, tl, loc, node, ntiles, group_sz):
    """Build a cross-core-uniform gather/scatter schedule.

    core/tl/loc/node: int arrays over items. Each item is gathered from
    pair-packed row (node>>1) and scatter-added to column `loc` of tile
    `tl` on core `core`.
    Returns (sched_dict, idx_i16 [NC,128,totidx//16], loc_f16 [NC,128,totchunks]).
    """
    HR = half_range()
    ngroups = (ntiles + group_sz - 1) // group_sz
    bucket = 2 * (node >= 2 * HR).astype(np.int64) + (node & 1)
    grp = tl // group_sz

    tid = ((core * ngroups + grp) * NBUCKET + bucket) * ntiles + tl
    n_bins = NC * ngroups * NBUCKET * ntiles
    cnt = np.bincount(tid, minlength=n_bins).reshape(NC, ngroups, NBUCKET, ntiles)
    K = (cnt + P - 1) // P
    K = K.max(axis=0)                       # [ngroups, NBUCKET, ntiles]
    per_tile = K.sum(axis=1)                # [ngroups, ntiles]
    for t in range(ntiles):
        g = t // group_sz
        if per_tile[g, t] == 0:
            K[g, 0, t] = 1

    chunk_meta = []
    win_meta = []
    for g in range(ngroups):
        t0, t1 = g * group_sz, min((g + 1) * group_sz, ntiles)
        for b in range(NBUCKET):
            c0 = len(chunk_meta)
            for t in range(t0, t1):
                for _ in range(K[g, b, t]):
                    chunk_meta.append((g, b, t))
            win_meta.append((g, b, c0, len(chunk_meta) - c0))
    totchunks = len(chunk_meta)
    totidx = totchunks * P

    slot_base = np.zeros((ngroups, NBUCKET, ntiles), np.int64)
    acc = 0
    for g in range(ngroups):
        t0, t1 = g * group_sz, min((g + 1) * group_sz, ntiles)
        for b in range(NBUCKET):
            for t in range(t0, t1):
                slot_base[g, b, t] = acc
                acc += K[g, b, t] * P
    assert acc == totidx

    loc_arr = np.full((NC, totidx), 255, np.int64)
    pidx_arr = np.zeros((NC, totidx), np.int64)
    order = np.lexsort((tl, bucket, grp, core))
    so_core = core[order]
    so_tid = tid[order]
    so_node = node[order]
    so_loc = loc[order]
    so_bucket = bucket[order]
    so_grp = grp[order]
    so_t = tl[order]
    starts = np.r_[0, np.flatnonzero(np.diff(so_tid)) + 1]
    run_ids = np.zeros(len(so_tid), np.int64)
    run_ids[starts[1:]] = 1
    run_ids = np.cumsum(run_ids)
    rank = np.arange(len(so_tid)) - starts[run_ids]
    slot = slot_base[so_grp, so_bucket, so_t] + rank
    pair_local = (so_node >> 1) - (so_bucket >= 2) * HR
    loc_arr[so_core, slot] = so_loc
    pidx_arr[so_core, slot] = pair_local

    loc_f16 = np.zeros((NC, P, totchunks), np.float16)
    idx_i16 = np.zeros((NC, P, totidx // 16), np.int16)
    for c in range(NC):
        loc_f16[c] = loc_arr[c].reshape(totchunks, P).T.astype(np.float16)
        for (g, b, c0, nch) in win_meta:
            if nch == 0:
                continue
            lo, hi = c0 * P, (c0 + nch) * P
            idx_i16[c][:, lo // 16: hi // 16] = _wrap_idx_window(pidx_arr[c, lo:hi])

    sched = {
        "chunk_meta": chunk_meta,
        "win_meta": win_meta,
        "totchunks": totchunks,
        "totidx": totidx,
        "ntiles": ntiles,
        "ngroups": ngroups,
        "group_sz": group_sz,
    }
    return sched, idx_i16, loc_f16


def _prep(inputs):
    x = np.asarray(inputs["x"], np.float32)
    edge_index = np.asarray(inputs["edge_index"], np.int64)
    index = np.asarray(inputs["index"], np.int64)
    W_conv = np.asarray(inputs["W_conv"], np.float32)
    b_conv = np.asarray(inputs["b_conv"], np.float32)
    W1 = np.asarray(inputs["W1"], np.float32)
    b1 = np.asarray(inputs["b1"], np.float32)
    W2 = np.asarray(inputs["W2"], np.float32)
    b2 = np.asarray(inputs["b2"], np.float32)

    n = x.shape[0]
    src = edge_index[0].astype(np.int64)
    dst = edge_index[1].astype(np.int64)
    loops = np.arange(n, dtype=np.int64)
    src = np.concatenate([src, loops])
    dst = np.concatenate([dst, loops])

    deg = np.bincount(dst, minlength=NPAD).astype(np.float32)
    deg[n:] = 1.0

    esched, eidx, eloc = _scatter_sched(
        core=dst // SHARD, tl=(dst % SHARD) // P, loc=dst % P, node=src,
        ntiles=TILES, group_sz=GROUP)

    # pair stream: per core PB pairs; side slots [xi: 0..PB) [xj: PB..2PB)
    B = index.shape[0]
    PB = B // NC
    assert PB % P == 0
    PCH = PB // P
    pair_global = np.arange(B, dtype=np.int64)
    pcore = pair_global // PB
    plocal = pair_global % PB
    s_core = np.concatenate([pcore, pcore])
    s_slot = np.concatenate([plocal, PB + plocal])
    s_node = np.concatenate([index[:, 0], index[:, 1]]).astype(np.int64)
    psched, pidx, ploc = _scatter_sched(
        core=s_core, tl=s_slot // P, loc=s_slot % P, node=s_node,
        ntiles=2 * PCH, group_sz=PGROUP)

    xpad = np.zeros((NPAD, F_IN), np.float32)
    xpad[:n] = x
    xT = xpad.T.astype(np.float16)
    xT_shards = [
        np.ascontiguousarray(
            xT[:, c * SHARD:(c + 1) * SHARD].reshape(2, P, SHARD).transpose(1, 0, 2)
        ) for c in range(NC)
    ]
    deg_sb = [
        np.ascontiguousarray(
            deg[c * SHARD:(c + 1) * SHARD].reshape(TILES, P).T
        ) for c in range(NC)
    ]

    consts = {
        "wc": np.ascontiguousarray(
            W_conv.reshape(2, P, H).transpose(1, 0, 2)).astype(np.float16),
        "bconvb": np.broadcast_to(b_conv, (P, H)).astype(np.float32).copy(),
        "iota": np.broadcast_to(np.arange(P, dtype=np.float16), (P, P)).copy(),
        "ident": np.eye(P, dtype=np.float16),
        "w1": W1.astype(np.float16),
        "b1": b1.reshape(16, 1).astype(np.float32),
        "w2": W2.astype(np.float32),
        "b2t": b2.reshape(1, 1).astype(np.float32),
    }
    sched = {"edge": esched, "pair": psched, "PCH": PCH}
    in_maps = []
    for c in range(NC):
        m = {
            "xt": xT_shards[c],
            "deg": deg_sb[c],
            "edstloc": eloc[c],
            "egidx": eidx[c],
            "pdstloc": ploc[c],
            "pgidx": pidx[c],
        }
        m.update(consts)
        in_maps.append(m)
    return in_maps, sched


def _emit_scatter(nc, dt, src_pairs, idx_dram, loc_sb, iota, sched,
                  pools, consume, prefix):
    """Gather pair-packed rows per window, build one-hot, matmul-accumulate
    per destination tile in PSUM, then hand each finished tile to consume."""
    widxp, msgp, ohp, accp = pools
    chunk_meta = sched["chunk_meta"]
    win_meta = sched["win_meta"]
    ngroups = sched["ngroups"]
    ntiles = sched["ntiles"]
    group_sz = sched["group_sz"]
    HR = half_range()

    first_chunk = {}
    last_chunk = {}
    for ci, (g, b, t) in enumerate(chunk_meta):
        first_chunk.setdefault(t, ci)
        last_chunk[t] = ci

    acc_tiles = {}
    for g in range(ngroups):
        t0, t1 = g * group_sz, min((g + 1) * group_sz, ntiles)
        for (gg, b, c0, nch) in win_meta:
            if gg != g or nch == 0:
                continue
            nidx = nch * P
            idxt = widxp.tile([P, nidx // 16], dt.int16, tag=f"{prefix}idx{b}",
                              name=f"{prefix}idx_g{g}b{b}")
            nc.sync.dma_start(
                idxt[:], idx_dram[:, c0 * P // 16:(c0 + nch) * P // 16])
            msg = msgp.tile([P, nch, P], dt.float16, tag=f"{prefix}msg{b}",
                            name=f"{prefix}msg_g{g}b{b}")
            src_ap = src_pairs if b < 2 else src_pairs[HR:, :]
            nc.gpsimd.dma_gather(
                msg[:], src_ap, idxt[:], nidx, nidx, P,
                single_packet=False, queue_num=0)
            oh = ohp.tile([P, nch, P], dt.float16, tag=f"{prefix}oh{b}",
                          name=f"{prefix}oh_g{g}b{b}")
            nc.vector.tensor_tensor(
                oh[:],
                loc_sb[:, c0:c0 + nch].unsqueeze(2).to_broadcast([P, nch, P]),
                iota[:, :].unsqueeze(1).to_broadcast([P, nch, P]),
                mybir.AluOpType.is_equal,
            )
            for ci in range(c0, c0 + nch):
                _, bb, t = chunk_meta[ci]
                if t not in acc_tiles:
                    acc_tiles[t] = accp.tile(
                        [P, H], dt.float32, tag=f"{prefix}acc{t % group_sz}",
                        name=f"{prefix}acc_t{t}")
                par = bb & 1
                nc.tensor.matmul(
                    acc_tiles[t][:],
                    lhsT=oh[:, ci - c0, :],
                    rhs=msg[:, ci - c0, par * H:(par + 1) * H],
                    start=(ci == first_chunk[t]),
                    stop=(ci == last_chunk[t]),
                )
        for t in range(t0, t1):
            consume(t, acc_tiles.pop(t))


def _build(sched, debug=False, passes=1):
    dt = mybir.dt
    esched = sched["edge"]
    psched = sched["pair"]
    PCH = sched["PCH"]

    nc = bacc.Bacc("TRN2", target_bir_lowering=False, debug=False,
                   enable_asserts=False, num_devices=NC, num_swdge_queues=4)

    xt_in = nc.dram_tensor("xt", [P, 2, SHARD], dt.float16, kind="ExternalInput")
    deg_in = nc.dram_tensor("deg", [P, TILES], dt.float32, kind="ExternalInput")
    edstloc_in = nc.dram_tensor("edstloc", [P, esched["totchunks"]], dt.float16,
                                kind="ExternalInput")
    egidx_in = nc.dram_tensor("egidx", [P, esched["totidx"] // 16], dt.int16,
                              kind="ExternalInput")
    pdstloc_in = nc.dram_tensor("pdstloc", [P, psched["totchunks"]], dt.float16,
                                kind="ExternalInput")
    pgidx_in = nc.dram_tensor("pgidx", [P, psched["totidx"] // 16], dt.int16,
                              kind="ExternalInput")
    wc_in = nc.dram_tensor("wc", [P, 2, H], dt.float16, kind="ExternalInput")
    bconvb_in = nc.dram_tensor("bconvb", [P, H], dt.float32, kind="ExternalInput")
    iota_in = nc.dram_tensor("iota", [P, P], dt.float16, kind="ExternalInput")
    ident_in = nc.dram_tensor("ident", [P, P], dt.float16, kind="ExternalInput")
    w1_in = nc.dram_tensor("w1", [P, 16], dt.float16, kind="ExternalInput")
    b1_in = nc.dram_tensor("b1", [16, 1], dt.float32, kind="ExternalInput")
    w2_in = nc.dram_tensor("w2", [16, 1], dt.float32, kind="ExternalInput")
    b2_in = nc.dram_tensor("b2t", [1, 1], dt.float32, kind="ExternalInput")
    outp = nc.dram_tensor("out", [PCH * P, 1], dt.float32, kind="ExternalOutput")
    if debug:
        dbg_g = nc.dram_tensor("dbg_g", [NPAD, H], dt.float16, kind="ExternalOutput")
        dbg_e = nc.dram_tensor("dbg_e", [NPAD, H], dt.float16, kind="ExternalOutput")

    g_shard = nc.dram_tensor("g_shard", [SHARD, H], dt.float16)
    g_full = nc.dram_tensor("g_full", [NPAD, H], dt.float16, addr_space="Shared")
    e_shard = nc.dram_tensor("e_shard", [SHARD, H], dt.float16)
    e_full = nc.dram_tensor("e_full", [NPAD, H], dt.float16)

    g_pairs = g_full[:, :].rearrange("(r two) f -> r (two f)", two=2)
    e_pairs = e_full[:, :].rearrange("(r two) f -> r (two f)", two=2)

    with tile.TileContext(nc) as tc:
        nc.gpsimd.load_library(library_config.mlp)

        with (
            tc.tile_pool(name="const", bufs=1) as cpool,
            tc.tile_pool(name="dinvp", bufs=1) as dpool,
        ):
            wc_sb = cpool.tile([P, 2, H], dt.float16)
            nc.sync.dma_start(wc_sb[:], wc_in[:, :, :])
            bconvb = cpool.tile([P, H], dt.float32)
            nc.sync.dma_start(bconvb[:], bconvb_in[:, :])
            iota = cpool.tile([P, P], dt.float16)
            nc.sync.dma_start(iota[:], iota_in[:, :])
            ident = cpool.tile([P, P], dt.float16)
            nc.sync.dma_start(ident[:], ident_in[:, :])
            w1_sb = cpool.tile([P, 16], dt.float16)
            nc.sync.dma_start(w1_sb[:], w1_in[:, :])
            b1_sb = cpool.tile([16, 1], dt.float32)
            nc.sync.dma_start(b1_sb[:], b1_in[:, :])
            w2_sb = cpool.tile([16, 1], dt.float32)
            nc.sync.dma_start(w2_sb[:], w2_in[:, :])
            b2_sb = cpool.tile([1, 1], dt.float32)
            nc.sync.dma_start(b2_sb[:], b2_in[:, :])
            edstloc_sb = cpool.tile([P, esched["totchunks"]], dt.float16)
            nc.sync.dma_start(edstloc_sb[:], edstloc_in[:, :])

            deg_sb = dpool.tile([P, TILES], dt.float32)
            nc.sync.dma_start(deg_sb[:], deg_in[:, :])
            sq = dpool.tile([P, TILES], dt.float32)
            nc.scalar.activation(sq[:], deg_sb[:], mybir.ActivationFunctionType.Sqrt)
            dinv = dpool.tile([P, TILES], dt.float32)
            nc.vector.reciprocal(dinv[:], sq[:])

            def _one_pass():
                # ---------------- phase A: g = (x @ W) * dinv ----------------
                XBLK = 16
                with (
                    tc.tile_pool(name="xtp", bufs=2) as xtp,
                    tc.tile_pool(name="hps", bufs=4, space="PSUM") as hps,
                    tc.tile_pool(name="gsb", bufs=1) as gsbp,
                ):
                    g_sb = gsbp.tile([P, TILES, H], dt.float16)
                    for blk in range((TILES + XBLK - 1) // XBLK):
                        t0, t1 = blk * XBLK, min((blk + 1) * XBLK, TILES)
                        xt_sb = xtp.tile([P, 2, (t1 - t0) * P], dt.float16, tag="xt")
                        nc.sync.dma_start(xt_sb[:], xt_in[:, :, t0 * P: t1 * P])
                        for t in range(t0, t1):
                            h_ps = hps.tile([P, H], dt.float32)
                            for k in range(2):
                                nc.tensor.matmul(
                                    h_ps[:],
                                    lhsT=xt_sb[:, k, (t - t0) * P:(t - t0 + 1) * P],
                                    rhs=wc_sb[:, k, :],
                                    start=(k == 0), stop=(k == 1),
                                )
                            nc.vector.tensor_scalar(
                                g_sb[:, t, :], h_ps[:], dinv[:, t:t + 1], None,
                                mybir.AluOpType.mult,
                            )
                    nc.sync.dma_start(
                        g_shard[:, :].rearrange("(t p) f -> p t f", p=P),
                        g_sb[:, :, :],
                    )

                nc.gpsimd.collective_compute(
                    "AllGather", mybir.AluOpType.bypass,
                    replica_groups=[list(range(NC))],
                    ins=[g_shard[:, :].opt()],
                    outs=[g_full[:, :].opt()],
                )

                # ---------------- phase C: aggregate per dst tile ----------------
                with (
                    tc.tile_pool(name="ewidx", bufs=2) as widxp,
                    tc.tile_pool(name="emsg", bufs=2) as msgp,
                    tc.tile_pool(name="eoh", bufs=2) as ohp,
                    tc.tile_pool(name="eacc", bufs=2, space="PSUM") as accp,
                    tc.tile_pool(name="epost", bufs=4) as postp,
                    tc.tile_pool(name="eemb", bufs=4) as embp,
                ):
                    def consume_edge(t, a):
                        e1 = postp.tile([P, H], dt.float32, tag="e1", name=f"e1_{t}")
                        nc.vector.tensor_scalar(
                            e1[:], a[:], dinv[:, t:t + 1], None, mybir.AluOpType.mult)
                        nc.vector.tensor_tensor(
                            e1[:], e1[:], bconvb[:], mybir.AluOpType.add)
                        m = postp.tile([P, H], dt.float32, tag="m", name=f"m_{t}")
                        nc.scalar.activation(
                            m[:], e1[:], mybir.ActivationFunctionType.Copy,
                            bias=0.0, scale=NEG)
                        emb = embp.tile([P, H], dt.float16, name=f"emb_{t}")
                        nc.vector.tensor_tensor(
                            emb[:], e1[:], m[:], mybir.AluOpType.max)
                        nc.sync.dma_start(e_shard[t * P:(t + 1) * P, :], emb[:])

                    _emit_scatter(nc, dt, g_pairs, egidx_in, edstloc_sb, iota,
                                  esched, (widxp, msgp, ohp, accp), consume_edge, "e")

                nc.gpsimd.collective_compute(
                    "AllGather", mybir.AluOpType.bypass,
                    replica_groups=[list(range(NC))],
                    ins=[e_shard[:, :].opt()],
                    outs=[e_full[:, :].opt()],
                )

                if debug:
                    nc.sync.dma_start(dbg_g[:, :], g_full[:, :])
                    nc.sync.dma_start(dbg_e[:, :], e_full[:, :])

                # ---------------- phase D: pair MLP ----------------
                with (
                    tc.tile_pool(name="pconst", bufs=1) as pcpool,
                    tc.tile_pool(name="pwidx", bufs=2) as pwidxp,
                    tc.tile_pool(name="pmsg", bufs=2) as pmsgp,
                    tc.tile_pool(name="poh", bufs=2) as pohp,
                    tc.tile_pool(name="pacc", bufs=1, space="PSUM") as paccp,
                    tc.tile_pool(name="pxs", bufs=1) as pxsp,
                    tc.tile_pool(name="ptps", bufs=2, space="PSUM") as ptps,
                    tc.tile_pool(name="pzps", bufs=1, space="PSUM") as pzps,
                    tc.tile_pool(name="pops", bufs=1, space="PSUM") as pops,
                    tc.tile_pool(name="psb", bufs=4) as psbp,
                ):
                    pdstloc_sb = pcpool.tile([P, psched["totchunks"]], dt.float16)
                    nc.sync.dma_start(pdstloc_sb[:], pdstloc_in[:, :])
                    xs_sb = pxsp.tile([P, psched["ntiles"], H], dt.float16)

                    def consume_pair(st, a):
                        nc.vector.tensor_copy(xs_sb[:, st, :], a[:])

                    _emit_scatter(nc, dt, e_pairs, pgidx_in, pdstloc_sb, iota,
                                  psched, (pwidxp, pmsgp, pohp, paccp),
                                  consume_pair, "p")

                    for k in range(PCH):
                        xt_ps = ptps.tile([P, P], dt.float16)
                        nc.tensor.transpose(xt_ps[0:H, :], xs_sb[:, k, :], ident[:])
                        nc.tensor.transpose(xt_ps[H:P, :], xs_sb[:, PCH + k, :], ident[:])
                        xijt = psbp.tile([P, P], dt.float16, tag="xijt")
                        nc.vector.tensor_copy(xijt[:], xt_ps[:])
                        z_ps = pzps.tile([16, P], dt.float32)
                        nc.tensor.matmul(z_ps[:], lhsT=w1_sb[:], rhs=xijt[:],
                                         start=True, stop=True)
                        zb = psbp.tile([16, P], dt.float32, tag="zb")
                        nc.vector.tensor_scalar(
                            zb[:], z_ps[:], b1_sb[:, 0:1], None, mybir.AluOpType.add)
                        m2 = psbp.tile([16, P], dt.float32, tag="m2")
                        nc.scalar.activation(
                            m2[:], zb[:], mybir.ActivationFunctionType.Copy,
                            bias=0.0, scale=NEG)
                        z2 = psbp.tile([16, P], dt.float32, tag="z2")
                        nc.vector.tensor_tensor(z2[:], zb[:], m2[:], mybir.AluOpType.max)
                        o_ps = pops.tile([1, P], dt.float32)
                        nc.tensor.matmul(o_ps[:], lhsT=w2_sb[:], rhs=z2[:],
                                         start=True, stop=True)
                        osb = psbp.tile([1, P], dt.float32, tag="osb")
                        nc.scalar.activation(
                            osb[:], o_ps[:], mybir.ActivationFunctionType.Sigmoid,
                            bias=b2_sb[:, 0:1], scale=1.0)
                        nc.sync.dma_start(
                            outp[k * P:(k + 1) * P, :].rearrange("r one -> one r"),
                            osb[0:1, :])

            for _ in range(passes):
                _one_pass()

    # align each gather's SWDGE queue with its Tile-assigned DMA lane so
    # semaphore<->queue locking stays consistent (4-way parallel desc gen)
    for blk in nc.m.functions[0].blocks:
        for inst in blk.instructions:
            if isinstance(inst, mybir.InstDMAGatherAnt):
                si = inst.sync_info
                for u in (si.on_update if si else []):
                    mm = re.match(r"DMASW(\d+)_", u.ant_name or "")
                    if mm:
                        inst.queue_num = int(mm.group(1)) % 4
                        break

    nc.compile()
    return nc


def kernel(**inputs) -> np.ndarray:
    in_maps, sched = _prep(inputs)
    nc = _build(sched)
    res = run_bass_kernel_spmd(nc, in_maps, list(range(NC)))
    out = np.concatenate([res.results[c]["out"] for c in range(NC)], axis=0)
    return out.astype(np.float32)


# revision 13
# speedup vs baseline: 1.2430x; 1.0060x over previous
"""GCN message-passing kernel for 8 Trainium2 NeuronCores (Bass/Tile).

Computes (matching the jax reference):
    h = x @ W_conv                      [N, H]
    node_embed = leaky_relu(D^-1/2 (A+I) D^-1/2 h + b_conv)
    out = sigmoid(leaky(cat(e[i], e[j]) @ W1 + b1) @ W2 + b2)

Distribution: nodes are sharded over the 8 cores (dst-sharded aggregation
with an AllGather of the scaled features g = dinv * h). Edges are
partitioned by destination shard and scatter-added on the TensorEngine
via per-destination-tile one-hot matmuls; per-edge source rows are
fetched with bulk dma_gather (pair-packed fp16 rows, 4 SWDGE queues).
The pair-MLP head reuses the same gather + one-hot-permute machinery to
collect node embeddings in pair order.
"""

import re

import numpy as np

import concourse.bass as bass
import concourse.bacc as bacc
import concourse.mybir as mybir
import concourse.tile as tile
from concourse import library_config
from concourse.bass_utils import run_bass_kernel_spmd

NC = 8
N_NODES = 100000
F_IN = 256
H = 64
NEG = 0.01

P = 128                    # partitions / tile height
TILES = 98                 # node tiles per core
SHARD = TILES * P          # 12544 nodes per core
NPAD = NC * SHARD          # 100352
GROUP = 4                  # node tiles per edge gather group
NBUCKET = 4                # (src range) x (src parity)
PGROUP = 4                 # pair slot-tiles per gather group


def half_range():
    return NPAD // 4       # 25088: int16-addressable pair rows per range


def _wrap_idx_window(idx):
    """int array [W] (W % 16 == 0) -> [128, W//16] int16 wrapped/replicated."""
    w = idx.reshape(-1, 16).T.astype(np.int16)
    return np.tile(w, (8, 1))


def _scatter_sched(core, tl, loc, node, ntiles, group_sz):
    """Build a cross-core-uniform gather/scatter schedule.

    core/tl/loc/node: int arrays over items. Each item is gathered from
    pair-packed row (node>>1) and scatter-added to column `loc` of tile
    `tl` on core `core`.
    Returns (sched_dict, idx_i16 [NC,128,totidx//16], loc_f16 [NC,128,totchunks]).
    """
    HR = half_range()
    ngroups = (ntiles + group_sz - 1) // group_sz
    bucket = 2 * (node >= 2 * HR).astype(np.int64) + (node & 1)
    grp = tl // group_sz

    tid = ((core * ngroups + grp) * NBUCKET + bucket) * ntiles + tl
    n_bins = NC * ngroups * NBUCKET * ntiles
    cnt = np.bincount(tid, minlength=n_bins).reshape(NC, ngroups, NBUCKET, ntiles)
    K = (cnt + P - 1) // P
    K = K.max(axis=0)                       # [ngroups, NBUCKET, ntiles]
    per_tile = K.sum(axis=1)                # [ngroups, ntiles]
    for t in range(ntiles):
        g = t // group_sz
        if per_tile[g, t] == 0:
            K[g, 0, t] = 1

    chunk_meta = []
    win_meta = []
    for g in range(ngroups):
        t0, t1 = g * group_sz, min((g + 1) * group_sz, ntiles)
        for b in range(NBUCKET):
            c0 = len(chunk_meta)
            for t in range(t0, t1):
                for _ in range(K[g, b, t]):
                    chunk_meta.append((g, b, t))
            win_meta.append((g, b, c0, len(chunk_meta) - c0))
    totchunks = len(chunk_meta)
    totidx = totchunks * P

    slot_base = np.zeros((ngroups, NBUCKET, ntiles), np.int64)
    acc = 0
    for g in range(ngroups):
        t0, t1 = g * group_sz, min((g + 1) * group_sz, ntiles)
        for b in range(NBUCKET):
            for t in range(t0, t1):
                slot_base[g, b, t] = acc
                acc += K[g, b, t] * P
    assert acc == totidx

    loc_arr = np.full((NC, totidx), 255, np.int64)
    pidx_arr = np.zeros((NC, totidx), np.int64)
    order = np.lexsort((tl, bucket, grp, core))
    so_core = core[order]
    so_tid = tid[order]
    so_node = node[order]
    so_loc = loc[order]
    so_bucket = bucket[order]
    so_grp = grp[order]
    so_t = tl[order]
    starts = np.r_[0, np.flatnonzero(np.diff(so_tid)) + 1]
    run_ids = np.zeros(len(so_tid), np.int64)
    run_ids[starts[1:]] = 1
    run_ids = np.cumsum(run_ids)
    rank = np.arange(len(so_tid)) - starts[run_ids]
    slot = slot_base[so_grp, so_bucket, so_t] + rank
    pair_local = (so_node >> 1) - (so_bucket >= 2) * HR
    loc_arr[so_core, slot] = so_loc
    pidx_arr[so_core, slot] = pair_local

    loc_f16 = np.zeros((NC, P, totchunks), np.float16)
    idx_i16 = np.zeros((NC, P, totidx // 16), np.int16)
    for c in range(NC):
        loc_f16[c] = loc_arr[c].reshape(totchunks, P).T.astype(np.float16)
        for (g, b, c0, nch) in win_meta:
            if nch == 0:
                continue
            lo, hi = c0 * P, (c0 + nch) * P
            idx_i16[c][:, lo // 16: hi // 16] = _wrap_idx_window(pidx_arr[c, lo:hi])

    sched = {
        "chunk_meta": chunk_meta,
        "win_meta": win_meta,
        "totchunks": totchunks,
        "totidx": totidx,
        "ntiles": ntiles,
        "ngroups": ngroups,
        "group_sz": group_sz,
    }
    return sched, idx_i16, loc_f16


def _prep(inputs):
    x = np.asarray(inputs["x"], np.float32)
    edge_index = np.asarray(inputs["edge_index"], np.int64)
    index = np.asarray(inputs["index"], np.int64)
    W_conv = np.asarray(inputs["W_conv"], np.float32)
    b_conv = np.asarray(inputs["b_conv"], np.float32)
    W1 = np.asarray(inputs["W1"], np.float32)
    b1 = np.asarray(inputs["b1"], np.float32)
    W2 = np.asarray(inputs["W2"], np.float32)
    b2 = np.asarray(inputs["b2"], np.float32)

    n = x.shape[0]
    src = edge_index[0].astype(np.int64)
    dst = edge_index[1].astype(np.int64)
    loops = np.arange(n, dtype=np.int64)
    src = np.concatenate([src, loops])
    dst = np.concatenate([dst, loops])

    deg = np.bincount(dst, minlength=NPAD).astype(np.float32)
    deg[n:] = 1.0

    esched, eidx, eloc = _scatter_sched(
        core=dst // SHARD, tl=(dst % SHARD) // P, loc=dst % P, node=src,
        ntiles=TILES, group_sz=GROUP)

    # pair stream: per core PB pairs; side slots [xi: 0..PB) [xj: PB..2PB)
    B = index.shape[0]
    PB = B // NC
    assert PB % P == 0
    PCH = PB // P
    pair_global = np.arange(B, dtype=np.int64)
    pcore = pair_global // PB
    plocal = pair_global % PB
    s_core = np.concatenate([pcore, pcore])
    s_slot = np.concatenate([plocal, PB + plocal])
    s_node = np.concatenate([index[:, 0], index[:, 1]]).astype(np.int64)
    psched, pidx, ploc = _scatter_sched(
        core=s_core, tl=s_slot // P, loc=s_slot % P, node=s_node,
        ntiles=2 * PCH, group_sz=PGROUP)

    xpad = np.zeros((NPAD, F_IN), np.float32)
    xpad[:n] = x
    xT = xpad.T.astype(np.float16)
    xT_shards = [
        np.ascontiguousarray(
            xT[:, c * SHARD:(c + 1) * SHARD].reshape(2, P, SHARD).transpose(1, 0, 2)
        ) for c in range(NC)
    ]
    deg_sb = [
        np.ascontiguousarray(
            deg[c * SHARD:(c + 1) * SHARD].reshape(TILES, P).T
        ) for c in range(NC)
    ]

    consts = {
        "wc": np.ascontiguousarray(
            W_conv.reshape(2, P, H).transpose(1, 0, 2)).astype(np.float16),
        "bconvb": np.broadcast_to(b_conv, (P, H)).astype(np.float32).copy(),
        "iota": np.broadcast_to(np.arange(P, dtype=np.float16), (P, P)).copy(),
        "ident": np.eye(P, dtype=np.float16),
        "w1": W1.astype(np.float16),
        "b1": b1.reshape(16, 1).astype(np.float32),
        "w2": W2.astype(np.float32),
        "b2t": b2.reshape(1, 1).astype(np.float32),
    }
    sched = {"edge": esched, "pair": psched, "PCH": PCH}
    in_maps = []
    for c in range(NC):
        m = {
            "xt": xT_shards[c],
            "deg": deg_sb[c],
            "edstloc": eloc[c],
            "egidx": eidx[c],
            "pdstloc": ploc[c],
            "pgidx": pidx[c],
        }
        m.update(consts)
        in_maps.append(m)
    return in_maps, sched


def _emit_scatter(nc, dt, src_pairs, idx_dram, loc_sb, iota, sched,
                  pools, consume, prefix, nidx_regs=None):
    """Gather pair-packed rows per window, build one-hot, matmul-accumulate
    per destination tile in PSUM, then hand each finished tile to consume."""
    widxp, msgp, ohp, accp = pools
    chunk_meta = sched["chunk_meta"]
    win_meta = sched["win_meta"]
    ngroups = sched["ngroups"]
    ntiles = sched["ntiles"]
    group_sz = sched["group_sz"]
    HR = half_range()

    first_chunk = {}
    last_chunk = {}
    for ci, (g, b, t) in enumerate(chunk_meta):
        first_chunk.setdefault(t, ci)
        last_chunk[t] = ci

    acc_tiles = {}
    for g in range(ngroups):
        t0, t1 = g * group_sz, min((g + 1) * group_sz, ntiles)
        for (gg, b, c0, nch) in win_meta:
            if gg != g or nch == 0:
                continue
            nidx = nch * P
            idxt = widxp.tile([P, nidx // 16], dt.int16, tag=f"{prefix}idx{b}",
                              name=f"{prefix}idx_g{g}b{b}")
            nc.sync.dma_start(
                idxt[:], idx_dram[:, c0 * P // 16:(c0 + nch) * P // 16])
            msg = msgp.tile([P, nch, P], dt.float16, tag=f"{prefix}msg{b}",
                            name=f"{prefix}msg_g{g}b{b}")
            src_ap = src_pairs if b < 2 else src_pairs[HR:, :]
            nc.gpsimd.dma_gather(
                msg[:], src_ap, idxt[:], nidx,
                nidx_regs[nidx] if nidx_regs else nidx, P,
                single_packet=False, queue_num=0)
            oh = ohp.tile([P, nch, P], dt.float16, tag=f"{prefix}oh{b}",
                          name=f"{prefix}oh_g{g}b{b}")
            nc.vector.tensor_tensor(
                oh[:],
                loc_sb[:, c0:c0 + nch].unsqueeze(2).to_broadcast([P, nch, P]),
                iota[:, :].unsqueeze(1).to_broadcast([P, nch, P]),
                mybir.AluOpType.is_equal,
            )
            for ci in range(c0, c0 + nch):
                _, bb, t = chunk_meta[ci]
                if t not in acc_tiles:
                    acc_tiles[t] = accp.tile(
                        [P, H], dt.float32, tag=f"{prefix}acc{t % group_sz}",
                        name=f"{prefix}acc_t{t}")
                par = bb & 1
                nc.tensor.matmul(
                    acc_tiles[t][:],
                    lhsT=oh[:, ci - c0, :],
                    rhs=msg[:, ci - c0, par * H:(par + 1) * H],
                    start=(ci == first_chunk[t]),
                    stop=(ci == last_chunk[t]),
                )
        for t in range(t0, t1):
            consume(t, acc_tiles.pop(t))


def _build(sched, debug=False, passes=1):
    dt = mybir.dt
    esched = sched["edge"]
    psched = sched["pair"]
    PCH = sched["PCH"]

    nc = bacc.Bacc("TRN2", target_bir_lowering=False, debug=False,
                   enable_asserts=False, num_devices=NC, num_swdge_queues=4)

    xt_in = nc.dram_tensor("xt", [P, 2, SHARD], dt.float16, kind="ExternalInput")
    deg_in = nc.dram_tensor("deg", [P, TILES], dt.float32, kind="ExternalInput")
    edstloc_in = nc.dram_tensor("edstloc", [P, esched["totchunks"]], dt.float16,
                                kind="ExternalInput")
    egidx_in = nc.dram_tensor("egidx", [P, esched["totidx"] // 16], dt.int16,
                              kind="ExternalInput")
    pdstloc_in = nc.dram_tensor("pdstloc", [P, psched["totchunks"]], dt.float16,
                                kind="ExternalInput")
    pgidx_in = nc.dram_tensor("pgidx", [P, psched["totidx"] // 16], dt.int16,
                              kind="ExternalInput")
    wc_in = nc.dram_tensor("wc", [P, 2, H], dt.float16, kind="ExternalInput")
    bconvb_in = nc.dram_tensor("bconvb", [P, H], dt.float32, kind="ExternalInput")
    iota_in = nc.dram_tensor("iota", [P, P], dt.float16, kind="ExternalInput")
    ident_in = nc.dram_tensor("ident", [P, P], dt.float16, kind="ExternalInput")
    w1_in = nc.dram_tensor("w1", [P, 16], dt.float16, kind="ExternalInput")
    b1_in = nc.dram_tensor("b1", [16, 1], dt.float32, kind="ExternalInput")
    w2_in = nc.dram_tensor("w2", [16, 1], dt.float32, kind="ExternalInput")
    b2_in = nc.dram_tensor("b2t", [1, 1], dt.float32, kind="ExternalInput")
    outp = nc.dram_tensor("out", [PCH * P, 1], dt.float32, kind="ExternalOutput")
    if debug:
        dbg_g = nc.dram_tensor("dbg_g", [NPAD, H], dt.float16, kind="ExternalOutput")
        dbg_e = nc.dram_tensor("dbg_e", [NPAD, H], dt.float16, kind="ExternalOutput")

    g_shard = nc.dram_tensor("g_shard", [SHARD, H], dt.float16)
    g_full = nc.dram_tensor("g_full", [NPAD, H], dt.float16, addr_space="Shared")
    e_shard = nc.dram_tensor("e_shard", [SHARD, H], dt.float16)
    e_full = nc.dram_tensor("e_full", [NPAD, H], dt.float16)

    g_pairs = g_full[:, :].rearrange("(r two) f -> r (two f)", two=2)
    e_pairs = e_full[:, :].rearrange("(r two) f -> r (two f)", two=2)

    with tile.TileContext(nc) as tc:
        nc.gpsimd.load_library(library_config.mlp)

        with (
            tc.tile_pool(name="const", bufs=1) as cpool,
            tc.tile_pool(name="dinvp", bufs=1) as dpool,
        ):
            wc_sb = cpool.tile([P, 2, H], dt.float16)
            nc.sync.dma_start(wc_sb[:], wc_in[:, :, :])
            bconvb = cpool.tile([P, H], dt.float32)
            nc.sync.dma_start(bconvb[:], bconvb_in[:, :])
            iota = cpool.tile([P, P], dt.float16)
            nc.sync.dma_start(iota[:], iota_in[:, :])
            ident = cpool.tile([P, P], dt.float16)
            nc.sync.dma_start(ident[:], ident_in[:, :])
            w1_sb = cpool.tile([P, 16], dt.float16)
            nc.sync.dma_start(w1_sb[:], w1_in[:, :])
            b1_sb = cpool.tile([16, 1], dt.float32)
            nc.sync.dma_start(b1_sb[:], b1_in[:, :])
            w2_sb = cpool.tile([16, 1], dt.float32)
            nc.sync.dma_start(w2_sb[:], w2_in[:, :])
            b2_sb = cpool.tile([1, 1], dt.float32)
            nc.sync.dma_start(b2_sb[:], b2_in[:, :])
            edstloc_sb = cpool.tile([P, esched["totchunks"]], dt.float16)
            nc.sync.dma_start(edstloc_sb[:], edstloc_in[:, :])

            # one shared register per distinct gather size: a fresh
            # immediate per gather serializes the Pool stream on a register
            # WAR hazard against the in-flight gather's num_idxs read
            sizes = sorted({nch * P
                            for sch in (esched, psched)
                            for (_, _, _, nch) in sch["win_meta"]
                            if nch > 0})
            nidx_regs = {v: nc.gpsimd.to_reg(v) for v in sizes}

            deg_sb = dpool.tile([P, TILES], dt.float32)
            nc.sync.dma_start(deg_sb[:], deg_in[:, :])
            sq = dpool.tile([P, TILES], dt.float32)
            nc.scalar.activation(sq[:], deg_sb[:], mybir.ActivationFunctionType.Sqrt)
            dinv = dpool.tile([P, TILES], dt.float32)
            nc.vector.reciprocal(dinv[:], sq[:])

            def _one_pass():
                # ---------------- phase A: g = (x @ W) * dinv ----------------
                XBLK = 16
                with (
                    tc.tile_pool(name="xtp", bufs=2) as xtp,
                    tc.tile_pool(name="hps", bufs=4, space="PSUM") as hps,
                    tc.tile_pool(name="gsb", bufs=1) as gsbp,
                ):
                    g_sb = gsbp.tile([P, TILES, H], dt.float16)
                    for blk in range((TILES + XBLK - 1) // XBLK):
                        t0, t1 = blk * XBLK, min((blk + 1) * XBLK, TILES)
                        xt_sb = xtp.tile([P, 2, (t1 - t0) * P], dt.float16, tag="xt")
                        nc.sync.dma_start(xt_sb[:], xt_in[:, :, t0 * P: t1 * P])
                        for t in range(t0, t1):
                            h_ps = hps.tile([P, H], dt.float32)
                            for k in range(2):
                                nc.tensor.matmul(
                                    h_ps[:],
                                    lhsT=xt_sb[:, k, (t - t0) * P:(t - t0 + 1) * P],
                                    rhs=wc_sb[:, k, :],
                                    start=(k == 0), stop=(k == 1),
                                )
                            nc.vector.tensor_scalar(
                                g_sb[:, t, :], h_ps[:], dinv[:, t:t + 1], None,
                                mybir.AluOpType.mult,
                            )
                    nc.sync.dma_start(
                        g_shard[:, :].rearrange("(t p) f -> p t f", p=P),
                        g_sb[:, :, :],
                    )

                nc.gpsimd.collective_compute(
                    "AllGather", mybir.AluOpType.bypass,
                    replica_groups=[list(range(NC))],
                    ins=[g_shard[:, :].opt()],
                    outs=[g_full[:, :].opt()],
                )

                # ---------------- phase C: aggregate per dst tile ----------------
                with (
                    tc.tile_pool(name="ewidx", bufs=4) as widxp,
                    tc.tile_pool(name="emsg", bufs=3) as msgp,
                    tc.tile_pool(name="eoh", bufs=3) as ohp,
                    tc.tile_pool(name="eacc", bufs=2, space="PSUM") as accp,
                    tc.tile_pool(name="epost", bufs=4) as postp,
                    tc.tile_pool(name="eemb", bufs=4) as embp,
                ):
                    def consume_edge(t, a):
                        e1 = postp.tile([P, H], dt.float32, tag="e1", name=f"e1_{t}")
                        nc.vector.tensor_scalar(
                            e1[:], a[:], dinv[:, t:t + 1], None, mybir.AluOpType.mult)
                        nc.vector.tensor_tensor(
                            e1[:], e1[:], bconvb[:], mybir.AluOpType.add)
                        m = postp.tile([P, H], dt.float32, tag="m", name=f"m_{t}")
                        nc.scalar.activation(
                            m[:], e1[:], mybir.ActivationFunctionType.Copy,
                            bias=0.0, scale=NEG)
                        emb = embp.tile([P, H], dt.float16, name=f"emb_{t}")
                        nc.vector.tensor_tensor(
                            emb[:], e1[:], m[:], mybir.AluOpType.max)
                        nc.sync.dma_start(e_shard[t * P:(t + 1) * P, :], emb[:])

                    _emit_scatter(nc, dt, g_pairs, egidx_in, edstloc_sb, iota,
                                  esched, (widxp, msgp, ohp, accp), consume_edge, "e",
                                  nidx_regs=nidx_regs)

                nc.gpsimd.collective_compute(
                    "AllGather", mybir.AluOpType.bypass,
                    replica_groups=[list(range(NC))],
                    ins=[e_shard[:, :].opt()],
                    outs=[e_full[:, :].opt()],
                )

                if debug:
                    nc.sync.dma_start(dbg_g[:, :], g_full[:, :])
                    nc.sync.dma_start(dbg_e[:, :], e_full[:, :])

                # ---------------- phase D: pair MLP ----------------
                with (
                    tc.tile_pool(name="pconst", bufs=1) as pcpool,
                    tc.tile_pool(name="pwidx", bufs=2) as pwidxp,
                    tc.tile_pool(name="pmsg", bufs=2) as pmsgp,
                    tc.tile_pool(name="poh", bufs=2) as pohp,
                    tc.tile_pool(name="pacc", bufs=1, space="PSUM") as paccp,
                    tc.tile_pool(name="pxs", bufs=1) as pxsp,
                    tc.tile_pool(name="ptps", bufs=2, space="PSUM") as ptps,
                    tc.tile_pool(name="pzps", bufs=1, space="PSUM") as pzps,
                    tc.tile_pool(name="pops", bufs=1, space="PSUM") as pops,
                    tc.tile_pool(name="psb", bufs=4) as psbp,
                ):
                    pdstloc_sb = pcpool.tile([P, psched["totchunks"]], dt.float16)
                    nc.sync.dma_start(pdstloc_sb[:], pdstloc_in[:, :])
                    xs_sb = pxsp.tile([P, psched["ntiles"], H], dt.float16)

                    def consume_pair(st, a):
                        nc.vector.tensor_copy(xs_sb[:, st, :], a[:])

                    _emit_scatter(nc, dt, e_pairs, pgidx_in, pdstloc_sb, iota,
                                  psched, (pwidxp, pmsgp, pohp, paccp),
                                  consume_pair, "p", nidx_regs=nidx_regs)

                    for k in range(PCH):
                        xt_ps = ptps.tile([P, P], dt.float16)
                        nc.tensor.transpose(xt_ps[0:H, :], xs_sb[:, k, :], ident[:])
                        nc.tensor.transpose(xt_ps[H:P, :], xs_sb[:, PCH + k, :], ident[:])
                        xijt = psbp.tile([P, P], dt.float16, tag="xijt")
                        nc.vector.tensor_copy(xijt[:], xt_ps[:])
                        z_ps = pzps.tile([16, P], dt.float32)
                        nc.tensor.matmul(z_ps[:], lhsT=w1_sb[:], rhs=xijt[:],
                                         start=True, stop=True)
                        zb = psbp.tile([16, P], dt.float32, tag="zb")
                        nc.vector.tensor_scalar(
                            zb[:], z_ps[:], b1_sb[:, 0:1], None, mybir.AluOpType.add)
                        m2 = psbp.tile([16, P], dt.float32, tag="m2")
                        nc.scalar.activation(
                            m2[:], zb[:], mybir.ActivationFunctionType.Copy,
                            bias=0.0, scale=NEG)
                        z2 = psbp.tile([16, P], dt.float32, tag="z2")
                        nc.vector.tensor_tensor(z2[:], zb[:], m2[:], mybir.AluOpType.max)
                        o_ps = pops.tile([1, P], dt.float32)
                        nc.tensor.matmul(o_ps[:], lhsT=w2_sb[:], rhs=z2[:],
                                         start=True, stop=True)
                        osb = psbp.tile([1, P], dt.float32, tag="osb")
                        nc.scalar.activation(
                            osb[:], o_ps[:], mybir.ActivationFunctionType.Sigmoid,
                            bias=b2_sb[:, 0:1], scale=1.0)
                        nc.sync.dma_start(
                            outp[k * P:(k + 1) * P, :].rearrange("r one -> one r"),
                            osb[0:1, :])

            for _ in range(passes):
                _one_pass()

    # align each gather's SWDGE queue with its Tile-assigned DMA lane so
    # semaphore<->queue locking stays consistent (4-way parallel desc gen)
    for blk in nc.m.functions[0].blocks:
        for inst in blk.instructions:
            if isinstance(inst, mybir.InstDMAGatherAnt):
                si = inst.sync_info
                for u in (si.on_update if si else []):
                    mm = re.match(r"DMASW(\d+)_", u.ant_name or "")
                    if mm:
                        inst.queue_num = int(mm.group(1)) % 4
                        break

    nc.compile()
    return nc


def kernel(**inputs) -> np.ndarray:
    in_maps, sched = _prep(inputs)
    nc = _build(sched)
    res = run_bass_kernel_spmd(nc, in_maps, list(range(NC)))
    out = np.concatenate([res.results[c]["out"] for c in range(NC)], axis=0)
    return out.astype(np.float32)


# revision 15
# speedup vs baseline: 1.3103x; 1.0542x over previous
"""GCN message-passing kernel for 8 Trainium2 NeuronCores (Bass/Tile).

Computes (matching the jax reference):
    h = x @ W_conv                      [N, H]
    node_embed = leaky_relu(D^-1/2 (A+I) D^-1/2 h + b_conv)
    out = sigmoid(leaky(cat(e[i], e[j]) @ W1 + b1) @ W2 + b2)

Distribution: nodes are sharded over the 8 cores (dst-sharded aggregation
with an AllGather of the scaled features g = dinv * h). Edges are
partitioned by destination shard and scatter-added on the TensorEngine
via per-destination-tile one-hot matmuls; per-edge source rows are
fetched with bulk dma_gather (pair-packed fp16 rows, 4 SWDGE queues).
The pair-MLP head reuses the same gather + one-hot-permute machinery to
collect node embeddings in pair order.
"""

import re

import numpy as np

import concourse.bass as bass
import concourse.bacc as bacc
import concourse.mybir as mybir
import concourse.tile as tile
from concourse import library_config
from concourse.bass_utils import run_bass_kernel_spmd

NC = 8
N_NODES = 100000
F_IN = 256
H = 64
NEG = 0.01

P = 128                    # partitions / tile height
TILES = 98                 # node tiles per core
SHARD = TILES * P          # 12544 nodes per core
NPAD = NC * SHARD          # 100352
GROUP = 4                  # node tiles per edge gather group
NBUCKET = 4                # (src range) x (src parity)
PGROUP = 4                 # pair slot-tiles per gather group


def half_range():
    return NPAD // 4       # 25088: int16-addressable pair rows per range


def _wrap_idx_window(idx):
    """int array [W] (W % 16 == 0) -> [128, W//16] int16 wrapped/replicated."""
    w = idx.reshape(-1, 16).T.astype(np.int16)
    return np.tile(w, (8, 1))


def _scatter_sched(core, tl, loc, node, ntiles, group_sz):
    """Build a cross-core-uniform gather/scatter schedule.

    core/tl/loc/node: int arrays over items. Each item is gathered from
    pair-packed row (node>>1) and scatter-added to column `loc` of tile
    `tl` on core `core`.
    Returns (sched_dict, idx_i16 [NC,128,totidx//16], loc_f16 [NC,128,totchunks]).
    """
    HR = half_range()
    ngroups = (ntiles + group_sz - 1) // group_sz
    bucket = 2 * (node >= 2 * HR).astype(np.int64) + (node & 1)
    grp = tl // group_sz

    tid = ((core * ngroups + grp) * NBUCKET + bucket) * ntiles + tl
    n_bins = NC * ngroups * NBUCKET * ntiles
    cnt = np.bincount(tid, minlength=n_bins).reshape(NC, ngroups, NBUCKET, ntiles)
    K = (cnt + P - 1) // P
    K = K.max(axis=0)                       # [ngroups, NBUCKET, ntiles]
    per_tile = K.sum(axis=1)                # [ngroups, ntiles]
    for t in range(ntiles):
        g = t // group_sz
        if per_tile[g, t] == 0:
            K[g, 0, t] = 1

    chunk_meta = []
    win_meta = []
    for g in range(ngroups):
        t0, t1 = g * group_sz, min((g + 1) * group_sz, ntiles)
        for b in range(NBUCKET):
            c0 = len(chunk_meta)
            for t in range(t0, t1):
                for _ in range(K[g, b, t]):
                    chunk_meta.append((g, b, t))
            win_meta.append((g, b, c0, len(chunk_meta) - c0))
    totchunks = len(chunk_meta)
    totidx = totchunks * P

    slot_base = np.zeros((ngroups, NBUCKET, ntiles), np.int64)
    acc = 0
    for g in range(ngroups):
        t0, t1 = g * group_sz, min((g + 1) * group_sz, ntiles)
        for b in range(NBUCKET):
            for t in range(t0, t1):
                slot_base[g, b, t] = acc
                acc += K[g, b, t] * P
    assert acc == totidx

    loc_arr = np.full((NC, totidx), 255, np.int64)
    pidx_arr = np.zeros((NC, totidx), np.int64)
    order = np.lexsort((tl, bucket, grp, core))
    so_core = core[order]
    so_tid = tid[order]
    so_node = node[order]
    so_loc = loc[order]
    so_bucket = bucket[order]
    so_grp = grp[order]
    so_t = tl[order]
    starts = np.r_[0, np.flatnonzero(np.diff(so_tid)) + 1]
    run_ids = np.zeros(len(so_tid), np.int64)
    run_ids[starts[1:]] = 1
    run_ids = np.cumsum(run_ids)
    rank = np.arange(len(so_tid)) - starts[run_ids]
    slot = slot_base[so_grp, so_bucket, so_t] + rank
    pair_local = (so_node >> 1) - (so_bucket >= 2) * HR
    loc_arr[so_core, slot] = so_loc
    pidx_arr[so_core, slot] = pair_local

    loc_f16 = np.zeros((NC, P, totchunks), np.float16)
    idx_i16 = np.zeros((NC, P, totidx // 16), np.int16)
    for c in range(NC):
        loc_f16[c] = loc_arr[c].reshape(totchunks, P).T.astype(np.float16)
        for (g, b, c0, nch) in win_meta:
            if nch == 0:
                continue
            lo, hi = c0 * P, (c0 + nch) * P
            idx_i16[c][:, lo // 16: hi // 16] = _wrap_idx_window(pidx_arr[c, lo:hi])

    sched = {
        "chunk_meta": chunk_meta,
        "win_meta": win_meta,
        "totchunks": totchunks,
        "totidx": totidx,
        "ntiles": ntiles,
        "ngroups": ngroups,
        "group_sz": group_sz,
    }
    return sched, idx_i16, loc_f16


def _prep(inputs):
    x = np.asarray(inputs["x"], np.float32)
    edge_index = np.asarray(inputs["edge_index"], np.int64)
    index = np.asarray(inputs["index"], np.int64)
    W_conv = np.asarray(inputs["W_conv"], np.float32)
    b_conv = np.asarray(inputs["b_conv"], np.float32)
    W1 = np.asarray(inputs["W1"], np.float32)
    b1 = np.asarray(inputs["b1"], np.float32)
    W2 = np.asarray(inputs["W2"], np.float32)
    b2 = np.asarray(inputs["b2"], np.float32)

    n = x.shape[0]
    src = edge_index[0].astype(np.int64)
    dst = edge_index[1].astype(np.int64)
    loops = np.arange(n, dtype=np.int64)
    src = np.concatenate([src, loops])
    dst = np.concatenate([dst, loops])

    deg = np.bincount(dst, minlength=NPAD).astype(np.float32)
    deg[n:] = 1.0

    esched, eidx, eloc = _scatter_sched(
        core=dst // SHARD, tl=(dst % SHARD) // P, loc=dst % P, node=src,
        ntiles=TILES, group_sz=GROUP)

    # pair stream: per core PB pairs; side slots [xi: 0..PB) [xj: PB..2PB)
    B = index.shape[0]
    PB = B // NC
    assert PB % P == 0
    PCH = PB // P
    pair_global = np.arange(B, dtype=np.int64)
    pcore = pair_global // PB
    plocal = pair_global % PB
    s_core = np.concatenate([pcore, pcore])
    s_slot = np.concatenate([plocal, PB + plocal])
    s_node = np.concatenate([index[:, 0], index[:, 1]]).astype(np.int64)
    psched, pidx, ploc = _scatter_sched(
        core=s_core, tl=s_slot // P, loc=s_slot % P, node=s_node,
        ntiles=2 * PCH, group_sz=PGROUP)

    xpad = np.zeros((NPAD, F_IN), np.float32)
    xpad[:n] = x
    xT = xpad.T.astype(np.float16)
    xT_shards = [
        np.ascontiguousarray(
            xT[:, c * SHARD:(c + 1) * SHARD].reshape(2, P, SHARD).transpose(1, 0, 2)
        ) for c in range(NC)
    ]
    deg_sb = [
        np.ascontiguousarray(
            deg[c * SHARD:(c + 1) * SHARD].reshape(TILES, P).T
        ) for c in range(NC)
    ]

    consts = {
        "wc": np.ascontiguousarray(
            W_conv.reshape(2, P, H).transpose(1, 0, 2)).astype(np.float16),
        "bconvb": np.broadcast_to(b_conv, (P, H)).astype(np.float32).copy(),
        "iota": np.broadcast_to(np.arange(P, dtype=np.float16), (P, P)).copy(),
        "ident": np.eye(P, dtype=np.float16),
        "w1": W1.astype(np.float16),
        "b1": b1.reshape(16, 1).astype(np.float32),
        "w2": W2.astype(np.float32),
        "b2t": b2.reshape(1, 1).astype(np.float32),
    }
    sched = {"edge": esched, "pair": psched, "PCH": PCH}
    in_maps = []
    for c in range(NC):
        m = {
            "xt": xT_shards[c],
            "deg": deg_sb[c],
            "edstloc": eloc[c],
            "egidx": eidx[c],
            "pdstloc": ploc[c],
            "pgidx": pidx[c],
        }
        m.update(consts)
        in_maps.append(m)
    return in_maps, sched


def _emit_scatter(nc, dt, src_pairs, idx_sb, loc_sb, iota, sched,
                  pools, consume, prefix, nidx_regs=None):
    """Gather pair-packed rows per window, build one-hot, matmul-accumulate
    per destination tile in PSUM, then hand each finished tile to consume."""
    msgp, ohp, accp = pools
    chunk_meta = sched["chunk_meta"]
    win_meta = sched["win_meta"]
    ngroups = sched["ngroups"]
    ntiles = sched["ntiles"]
    group_sz = sched["group_sz"]
    HR = half_range()

    first_chunk = {}
    last_chunk = {}
    for ci, (g, b, t) in enumerate(chunk_meta):
        first_chunk.setdefault(t, ci)
        last_chunk[t] = ci

    acc_tiles = {}
    for g in range(ngroups):
        t0, t1 = g * group_sz, min((g + 1) * group_sz, ntiles)
        for (gg, b, c0, nch) in win_meta:
            if gg != g or nch == 0:
                continue
            nidx = nch * P
            msg = msgp.tile([P, nch, P], dt.float16, tag=f"{prefix}msg{b}",
                            name=f"{prefix}msg_g{g}b{b}")
            src_ap = src_pairs if b < 2 else src_pairs[HR:, :]
            nc.gpsimd.dma_gather(
                msg[:], src_ap,
                idx_sb[:, c0 * P // 16:(c0 + nch) * P // 16], nidx,
                nidx_regs[nidx] if nidx_regs else nidx, P,
                single_packet=False, queue_num=0)
            oh = ohp.tile([P, nch, P], dt.float16, tag=f"{prefix}oh{b}",
                          name=f"{prefix}oh_g{g}b{b}")
            nc.vector.tensor_tensor(
                oh[:],
                loc_sb[:, c0:c0 + nch].unsqueeze(2).to_broadcast([P, nch, P]),
                iota[:, :].unsqueeze(1).to_broadcast([P, nch, P]),
                mybir.AluOpType.is_equal,
            )
            for ci in range(c0, c0 + nch):
                _, bb, t = chunk_meta[ci]
                if t not in acc_tiles:
                    acc_tiles[t] = accp.tile(
                        [P, H], dt.float32, tag=f"{prefix}acc{t % group_sz}",
                        name=f"{prefix}acc_t{t}")
                par = bb & 1
                nc.tensor.matmul(
                    acc_tiles[t][:],
                    lhsT=oh[:, ci - c0, :],
                    rhs=msg[:, ci - c0, par * H:(par + 1) * H],
                    start=(ci == first_chunk[t]),
                    stop=(ci == last_chunk[t]),
                )
        for t in range(t0, t1):
            consume(t, acc_tiles.pop(t))


def _build(sched, debug=False, passes=1):
    dt = mybir.dt
    esched = sched["edge"]
    psched = sched["pair"]
    PCH = sched["PCH"]

    nc = bacc.Bacc("TRN2", target_bir_lowering=False, debug=False,
                   enable_asserts=False, num_devices=NC, num_swdge_queues=4)

    xt_in = nc.dram_tensor("xt", [P, 2, SHARD], dt.float16, kind="ExternalInput")
    deg_in = nc.dram_tensor("deg", [P, TILES], dt.float32, kind="ExternalInput")
    edstloc_in = nc.dram_tensor("edstloc", [P, esched["totchunks"]], dt.float16,
                                kind="ExternalInput")
    egidx_in = nc.dram_tensor("egidx", [P, esched["totidx"] // 16], dt.int16,
                              kind="ExternalInput")
    pdstloc_in = nc.dram_tensor("pdstloc", [P, psched["totchunks"]], dt.float16,
                                kind="ExternalInput")
    pgidx_in = nc.dram_tensor("pgidx", [P, psched["totidx"] // 16], dt.int16,
                              kind="ExternalInput")
    wc_in = nc.dram_tensor("wc", [P, 2, H], dt.float16, kind="ExternalInput")
    bconvb_in = nc.dram_tensor("bconvb", [P, H], dt.float32, kind="ExternalInput")
    iota_in = nc.dram_tensor("iota", [P, P], dt.float16, kind="ExternalInput")
    ident_in = nc.dram_tensor("ident", [P, P], dt.float16, kind="ExternalInput")
    w1_in = nc.dram_tensor("w1", [P, 16], dt.float16, kind="ExternalInput")
    b1_in = nc.dram_tensor("b1", [16, 1], dt.float32, kind="ExternalInput")
    w2_in = nc.dram_tensor("w2", [16, 1], dt.float32, kind="ExternalInput")
    b2_in = nc.dram_tensor("b2t", [1, 1], dt.float32, kind="ExternalInput")
    outp = nc.dram_tensor("out", [PCH * P, 1], dt.float32, kind="ExternalOutput")
    if debug:
        dbg_g = nc.dram_tensor("dbg_g", [NPAD, H], dt.float16, kind="ExternalOutput")
        dbg_e = nc.dram_tensor("dbg_e", [NPAD, H], dt.float16, kind="ExternalOutput")

    g_shard = nc.dram_tensor("g_shard", [SHARD, H], dt.float16)
    g_full = nc.dram_tensor("g_full", [NPAD, H], dt.float16, addr_space="Shared")
    e_shard = nc.dram_tensor("e_shard", [SHARD, H], dt.float16)
    e_full = nc.dram_tensor("e_full", [NPAD, H], dt.float16)

    g_pairs = g_full[:, :].rearrange("(r two) f -> r (two f)", two=2)
    e_pairs = e_full[:, :].rearrange("(r two) f -> r (two f)", two=2)

    with tile.TileContext(nc) as tc:
        nc.gpsimd.load_library(library_config.mlp)

        with (
            tc.tile_pool(name="const", bufs=1) as cpool,
            tc.tile_pool(name="dinvp", bufs=1) as dpool,
        ):
            wc_sb = cpool.tile([P, 2, H], dt.float16)
            nc.sync.dma_start(wc_sb[:], wc_in[:, :, :])
            bconvb = cpool.tile([P, H], dt.float32)
            nc.sync.dma_start(bconvb[:], bconvb_in[:, :])
            iota = cpool.tile([P, P], dt.float16)
            nc.sync.dma_start(iota[:], iota_in[:, :])
            ident = cpool.tile([P, P], dt.float16)
            nc.sync.dma_start(ident[:], ident_in[:, :])
            w1_sb = cpool.tile([P, 16], dt.float16)
            nc.sync.dma_start(w1_sb[:], w1_in[:, :])
            b1_sb = cpool.tile([16, 1], dt.float32)
            nc.sync.dma_start(b1_sb[:], b1_in[:, :])
            w2_sb = cpool.tile([16, 1], dt.float32)
            nc.sync.dma_start(w2_sb[:], w2_in[:, :])
            b2_sb = cpool.tile([1, 1], dt.float32)
            nc.sync.dma_start(b2_sb[:], b2_in[:, :])
            edstloc_sb = cpool.tile([P, esched["totchunks"]], dt.float16)
            nc.sync.dma_start(edstloc_sb[:], edstloc_in[:, :])
            egidx_sb = cpool.tile([P, esched["totidx"] // 16], dt.int16)
            nc.sync.dma_start(egidx_sb[:], egidx_in[:, :])
            pgidx_sb = cpool.tile([P, psched["totidx"] // 16], dt.int16)
            nc.sync.dma_start(pgidx_sb[:], pgidx_in[:, :])

            # one shared register per distinct gather size: a fresh
            # immediate per gather serializes the Pool stream on a register
            # WAR hazard against the in-flight gather's num_idxs read
            sizes = sorted({nch * P
                            for sch in (esched, psched)
                            for (_, _, _, nch) in sch["win_meta"]
                            if nch > 0})
            nidx_regs = {v: nc.gpsimd.to_reg(v) for v in sizes}

            deg_sb = dpool.tile([P, TILES], dt.float32)
            nc.sync.dma_start(deg_sb[:], deg_in[:, :])
            sq = dpool.tile([P, TILES], dt.float32)
            nc.scalar.activation(sq[:], deg_sb[:], mybir.ActivationFunctionType.Sqrt)
            dinv = dpool.tile([P, TILES], dt.float32)
            nc.vector.reciprocal(dinv[:], sq[:])

            def _one_pass():
                # ---------------- phase A: g = (x @ W) * dinv ----------------
                XBLK = 16
                with (
                    tc.tile_pool(name="xtp", bufs=2) as xtp,
                    tc.tile_pool(name="hps", bufs=4, space="PSUM") as hps,
                    tc.tile_pool(name="gsb", bufs=1) as gsbp,
                ):
                    g_sb = gsbp.tile([P, TILES, H], dt.float16)
                    for blk in range((TILES + XBLK - 1) // XBLK):
                        t0, t1 = blk * XBLK, min((blk + 1) * XBLK, TILES)
                        xt_sb = xtp.tile([P, 2, (t1 - t0) * P], dt.float16, tag="xt")
                        nc.sync.dma_start(xt_sb[:], xt_in[:, :, t0 * P: t1 * P])
                        for t in range(t0, t1):
                            h_ps = hps.tile([P, H], dt.float32)
                            for k in range(2):
                                nc.tensor.matmul(
                                    h_ps[:],
                                    lhsT=xt_sb[:, k, (t - t0) * P:(t - t0 + 1) * P],
                                    rhs=wc_sb[:, k, :],
                                    start=(k == 0), stop=(k == 1),
                                )
                            nc.vector.tensor_scalar(
                                g_sb[:, t, :], h_ps[:], dinv[:, t:t + 1], None,
                                mybir.AluOpType.mult,
                            )
                    nc.sync.dma_start(
                        g_shard[:, :].rearrange("(t p) f -> p t f", p=P),
                        g_sb[:, :, :],
                    )

                nc.gpsimd.collective_compute(
                    "AllGather", mybir.AluOpType.bypass,
                    replica_groups=[list(range(NC))],
                    ins=[g_shard[:, :].opt()],
                    outs=[g_full[:, :].opt()],
                )

                # ---------------- phase C: aggregate per dst tile ----------------
                with (
                    tc.tile_pool(name="emsg", bufs=3) as msgp,
                    tc.tile_pool(name="eoh", bufs=3) as ohp,
                    tc.tile_pool(name="eacc", bufs=2, space="PSUM") as accp,
                    tc.tile_pool(name="epost", bufs=4) as postp,
                    tc.tile_pool(name="eemb", bufs=4) as embp,
                ):
                    def consume_edge(t, a):
                        e1 = postp.tile([P, H], dt.float32, tag="e1", name=f"e1_{t}")
                        nc.vector.tensor_scalar(
                            e1[:], a[:], dinv[:, t:t + 1], None, mybir.AluOpType.mult)
                        nc.vector.tensor_tensor(
                            e1[:], e1[:], bconvb[:], mybir.AluOpType.add)
                        m = postp.tile([P, H], dt.float32, tag="m", name=f"m_{t}")
                        nc.scalar.activation(
                            m[:], e1[:], mybir.ActivationFunctionType.Copy,
                            bias=0.0, scale=NEG)
                        emb = embp.tile([P, H], dt.float16, tag="emb",
                                        name=f"emb_{t}")
                        nc.vector.tensor_tensor(
                            emb[:], e1[:], m[:], mybir.AluOpType.max)
                        nc.sync.dma_start(e_shard[t * P:(t + 1) * P, :], emb[:])

                    _emit_scatter(nc, dt, g_pairs, egidx_sb, edstloc_sb, iota,
                                  esched, (msgp, ohp, accp), consume_edge, "e",
                                  nidx_regs=nidx_regs)

                nc.gpsimd.collective_compute(
                    "AllGather", mybir.AluOpType.bypass,
                    replica_groups=[list(range(NC))],
                    ins=[e_shard[:, :].opt()],
                    outs=[e_full[:, :].opt()],
                )

                if debug:
                    nc.sync.dma_start(dbg_g[:, :], g_full[:, :])
                    nc.sync.dma_start(dbg_e[:, :], e_full[:, :])

                # ---------------- phase D: pair MLP ----------------
                with (
                    tc.tile_pool(name="pconst", bufs=1) as pcpool,
                    tc.tile_pool(name="pmsg", bufs=2) as pmsgp,
                    tc.tile_pool(name="poh", bufs=2) as pohp,
                    tc.tile_pool(name="pacc", bufs=1, space="PSUM") as paccp,
                    tc.tile_pool(name="pxs", bufs=1) as pxsp,
                    tc.tile_pool(name="ptps", bufs=2, space="PSUM") as ptps,
                    tc.tile_pool(name="pzps", bufs=1, space="PSUM") as pzps,
                    tc.tile_pool(name="pops", bufs=1, space="PSUM") as pops,
                    tc.tile_pool(name="psb", bufs=4) as psbp,
                ):
                    pdstloc_sb = pcpool.tile([P, psched["totchunks"]], dt.float16)
                    nc.sync.dma_start(pdstloc_sb[:], pdstloc_in[:, :])
                    xs_sb = pxsp.tile([P, psched["ntiles"], H], dt.float16)

                    def consume_pair(st, a):
                        nc.vector.tensor_copy(xs_sb[:, st, :], a[:])

                    _emit_scatter(nc, dt, e_pairs, pgidx_sb, pdstloc_sb, iota,
                                  psched, (pmsgp, pohp, paccp),
                                  consume_pair, "p", nidx_regs=nidx_regs)

                    for k in range(PCH):
                        xt_ps = ptps.tile([P, P], dt.float16)
                        nc.tensor.transpose(xt_ps[0:H, :], xs_sb[:, k, :], ident[:])
                        nc.tensor.transpose(xt_ps[H:P, :], xs_sb[:, PCH + k, :], ident[:])
                        xijt = psbp.tile([P, P], dt.float16, tag="xijt")
                        nc.vector.tensor_copy(xijt[:], xt_ps[:])
                        z_ps = pzps.tile([16, P], dt.float32)
                        nc.tensor.matmul(z_ps[:], lhsT=w1_sb[:], rhs=xijt[:],
                                         start=True, stop=True)
                        zb = psbp.tile([16, P], dt.float32, tag="zb")
                        nc.vector.tensor_scalar(
                            zb[:], z_ps[:], b1_sb[:, 0:1], None, mybir.AluOpType.add)
                        m2 = psbp.tile([16, P], dt.float32, tag="m2")
                        nc.scalar.activation(
                            m2[:], zb[:], mybir.ActivationFunctionType.Copy,
                            bias=0.0, scale=NEG)
                        z2 = psbp.tile([16, P], dt.float32, tag="z2")
                        nc.vector.tensor_tensor(z2[:], zb[:], m2[:], mybir.AluOpType.max)
                        o_ps = pops.tile([1, P], dt.float32)
                        nc.tensor.matmul(o_ps[:], lhsT=w2_sb[:], rhs=z2[:],
                                         start=True, stop=True)
                        osb = psbp.tile([1, P], dt.float32, tag="osb")
                        nc.scalar.activation(
                            osb[:], o_ps[:], mybir.ActivationFunctionType.Sigmoid,
                            bias=b2_sb[:, 0:1], scale=1.0)
                        nc.sync.dma_start(
                            outp[k * P:(k + 1) * P, :].rearrange("r one -> one r"),
                            osb[0:1, :])

            for _ in range(passes):
                _one_pass()

    # align each gather's SWDGE queue with its Tile-assigned DMA lane so
    # semaphore<->queue locking stays consistent (4-way parallel desc gen)
    for blk in nc.m.functions[0].blocks:
        for inst in blk.instructions:
            if isinstance(inst, mybir.InstDMAGatherAnt):
                si = inst.sync_info
                for u in (si.on_update if si else []):
                    mm = re.match(r"DMASW(\d+)_", u.ant_name or "")
                    if mm:
                        inst.queue_num = int(mm.group(1)) % 4
                        break

    nc.compile()
    return nc


def kernel(**inputs) -> np.ndarray:
    in_maps, sched = _prep(inputs)
    nc = _build(sched)
    res = run_bass_kernel_spmd(nc, in_maps, list(range(NC)))
    out = np.concatenate([res.results[c]["out"] for c in range(NC)], axis=0)
    return out.astype(np.float32)
